# revision 35
# baseline (speedup 1.0000x reference)
"""GATv2 (2-layer, 2-head) Trainium2 kernel, 8-core SPMD — v5.

vs v4: layer-1 xl table computed redundantly on every core (no L1
AllGather, fast startup), conv batches software-pipelined (u-phase of
batch b+1 issues before score/y/scatter of batch b so PE never waits
behind the DVE/ACT chain), DMA loads prefetch 2 batches ahead, leaky
back to single scalar_tensor_tensor, y fully on DVE.
"""
import sys

sys.path.insert(0, "/opt/trn_rl_repo")

import numpy as np
import ml_dtypes

BF = ml_dtypes.bfloat16

# ---- static layout constants (match reference problem sizes) ----
N = 50000
NCORES = 8
LANES = 128
NTILES = 49
SPC = NTILES * LANES          # 6272 slots per core
S = NCORES * SPC              # 50176 total slots
GTILES = S // 128             # 392 gather-table tiles
TA = 7                        # table-A gather subtiles per dst-tile
TB = 7
TS = TA + TB                  # random-edge subtiles (self subtile is extra)
NS = TS + 1                   # subtiles per tile incl self
GB = 3                        # dst-tiles per gather batch
IN_F = 128
HC = 256                      # H*C
OUT_F = 40
SLOPE = 0.2
# AllGather chunking (layer 2 only): 5 tile groups (sum = NTILES).
# Groups 0,1 make up gather table A, groups 2,3,4 table B.
AG_CH = (16, 8, 12, 8, 5)
AG_T0 = (0, 16, 24, 36, 44)
AG_TAB = (0, 0, 1, 1, 1)
NTILES_A = 24                 # tiles in table A
HALFR = NTILES_A * LANES * NCORES   # 24576 rows in table A
_b = [0] * len(AG_CH)
_acc = [0, 0]
for _c in range(len(AG_CH)):
    _b[_c] = (0 if AG_TAB[_c] == 0 else HALFR) + _acc[AG_TAB[_c]]
    _acc[AG_TAB[_c]] += AG_CH[_c] * LANES * NCORES
AG_BASE = tuple(_b)

_NC_CACHE = {}
_RUN_OPTS = {}
_LAST_RESULTS = {}
_LR_RANGES = {}


# ---------------------------------------------------------------- host prep
def _pack_graph(src, dst):
    deg = np.bincount(dst, minlength=N)

    is_self = src == dst
    self_eids = np.full(N, -1, np.int64)
    sids = np.where(is_self)[0]
    self_eids[src[sids]] = sids
    rand_mask = np.ones(len(src), bool)
    rand_mask[self_eids[self_eids >= 0]] = False

    nodes_per_core = (N + NCORES - 1) // NCORES
    order = np.argsort(-deg, kind="stable")
    core_edges = np.zeros(NCORES, np.int64)
    core_nodes = np.zeros(NCORES, np.int64)
    core_of_node = np.full(N, -1, np.int32)
    for v in order:
        k = np.argmin(np.where(core_nodes < nodes_per_core, core_edges, 1 << 60))
        core_of_node[v] = k
        core_edges[k] += deg[v]
        core_nodes[k] += 1

    rsrc, rdst = src[rand_mask], dst[rand_mask]

    # --- chunk-group assignment per core (before tile packing): deal nodes
    # round-robin by out-degree so the gather-table halves stay balanced.
    NG = len(AG_CH)
    odeg = np.bincount(rsrc, minlength=N)
    group_of_node = np.full(N, -1, np.int8)
    gcap = [c * LANES for c in AG_CH]
    for k in range(NCORES):
        vs = np.where(core_of_node == k)[0]
        vs = vs[np.argsort(-odeg[vs], kind="stable")]
        cnt = [0] * NG
        gi = 0
        for v in vs:
            while cnt[gi % NG] >= gcap[gi % NG]:
                gi += 1
            group_of_node[v] = gi % NG
            cnt[gi % NG] += 1
            gi += 1
    eh_node = np.asarray(AG_TAB, np.int8)[group_of_node]

    dA = np.bincount(rdst[eh_node[rsrc] == 0], minlength=N)
    dB = np.bincount(rdst[eh_node[rsrc] == 1], minlength=N)
    capA, capB = TA * LANES, TB * LANES

    tile_of_node = np.full(N, -1, np.int32)
    lane_of_node = np.full(N, -1, np.int32)
    for k in range(NCORES):
        for g in range(NG):
            vs = np.where((core_of_node == k) & (group_of_node == g))[0]
            vs = vs[np.argsort(-(dA[vs] + dB[vs]), kind="stable")]
            nv = len(vs)
            ntg = AG_CH[g]
            tile = np.empty(nv, np.int64)
            for i in range(nv):
                r, c = divmod(i, ntg)
                tile[i] = c if r % 2 == 0 else ntg - 1 - c
            loadA = np.bincount(tile, weights=dA[vs],
                                minlength=ntg).astype(np.int64)
            loadB = np.bincount(tile, weights=dB[vs],
                                minlength=ntg).astype(np.int64)
            it = 0
            while (loadA.max() > capA or loadB.max() > capB) and it < 100000:
                it += 1
                t_bad = int(np.argmax(np.maximum(loadA - capA, loadB - capB)))
                overA = loadA[t_bad] - capA >= loadB[t_bad] - capB
                t_good = int(np.argmin(loadA + loadB))
                in_bad = np.where(tile == t_bad)[0]
                in_good = np.where(tile == t_good)[0]
                d_bad = dA[vs[in_bad]] if overA else dB[vs[in_bad]]
                ib = in_bad[np.argmax(d_bad)]
                ig = in_good[np.argmin(dA[vs[in_good]] + dB[vs[in_good]])]
                for i, frm, to in ((ib, t_bad, t_good), (ig, t_good, t_bad)):
                    v = vs[i]
                    tile[i] = to
                    loadA[frm] -= dA[v]; loadA[to] += dA[v]
                    loadB[frm] -= dB[v]; loadB[to] += dB[v]
            if loadA.max() > capA or loadB.max() > capB:
                raise RuntimeError("edge packing failed; need bigger TA/TB")
            tile_of_node[vs] = AG_T0[g] + tile
            for t in range(ntg):
                nodes_t = vs[tile == t]
                lane_of_node[nodes_t] = np.arange(len(nodes_t))

    slot_of_node = (core_of_node.astype(np.int64) * SPC
                    + tile_of_node * LANES + lane_of_node)
    node_of_slot = np.full(S, -1, np.int64)
    node_of_slot[slot_of_node] = np.arange(N)

    # chunk-major gather-table row of each node
    g_arr = group_of_node.astype(np.int64)
    base = np.asarray(AG_BASE, np.int64)[g_arr]
    t0 = np.asarray(AG_T0, np.int64)[g_arr]
    chw = np.asarray(AG_CH, np.int64)[g_arr]
    grow_of_node = (base + core_of_node * chw * LANES
                    + (tile_of_node - t0) * LANES + lane_of_node)

    srcrow = grow_of_node[rsrc]
    dstslot = slot_of_node[rdst]
    dst_core = (dstslot // SPC).astype(np.int32)
    dst_tile = ((dstslot % SPC) // LANES).astype(np.int32)
    dst_lane = (dstslot % LANES).astype(np.int32)
    eh = (srcrow >= HALFR).astype(np.int8)

    idxXL = np.zeros((NCORES, NTILES, TS * 128), np.int16)
    idxXL1 = np.zeros((NCORES, NTILES, TS * 128), np.int16)

    key = (dst_core.astype(np.int64) * NTILES + dst_tile) * 2 + eh
    es = np.argsort(key, kind="stable")
    ksrc = srcrow[es]; kdl = dst_lane[es]
    kc = dst_core[es]; kt = dst_tile[es]; kh = eh[es]
    gkey = key[es]
    start = np.zeros(len(es), bool)
    start[0] = True
    start[1:] = gkey[1:] != gkey[:-1]
    gs = np.where(start, np.arange(len(es)), 0)
    gidx = np.arange(len(es)) - np.maximum.accumulate(gs)
    off = np.where(kh == 0, 0, TA * 128) + gidx
    tabrow = np.where(kh == 0, ksrc, ksrc - HALFR).astype(np.int64)
    idxXL[kc, kt, off] = tabrow.astype(np.int16)
    # conv1 gathers read the lane-major replicated L1 table:
    # row' = lane*(tiles in table) + tile
    ntt = np.where(kh == 0, HALFR // 128, (S - HALFR) // 128)
    row1 = (tabrow % 128) * ntt + tabrow // 128
    idxXL1[kc, kt, off] = row1.astype(np.int16)

    # one-hot masks: mk [e-lane -> dst-lane] per subtile (incl self at TS),
    # mkT [dst-lane -> e-lane] per random subtile.
    ksi = (off // 128).astype(np.int64)
    kel = (off % 128).astype(np.int64)
    mk = np.zeros((NCORES, NTILES, 128, NS * 128), np.float32)
    mkT = np.zeros((NCORES, NTILES, 128, TS * 128), np.float32)
    mk[kc, kt, kel, ksi * 128 + kdl] = 1.0
    mkT[kc, kt, kdl, ksi * 128 + kel] = 1.0
    vsel = np.where(self_eids >= 0)[0]
    ln = lane_of_node[vsel].astype(np.int64)
    mk[core_of_node[vsel], tile_of_node[vsel], ln, TS * 128 + ln] = 1.0

    # grow-order slot map (for the replicated L1 table build)
    slot_of_grow = np.full(S, -1, np.int64)
    slot_of_grow[grow_of_node[np.arange(N)]] = slot_of_node

    return dict(slot_of_node=slot_of_node, node_of_slot=node_of_slot,
                idxXL=idxXL, idxXL1=idxXL1, mk=mk, mkT=mkT,
                slot_of_grow=slot_of_grow)


def _wrap_idx(idx):
    """[n] -> [128, n//16] wrapped (j at partition j%16, col j//16) + replicated."""
    n = idx.shape[0]
    a = idx.reshape(n // 16, 16).T.astype(np.int16)
    return np.tile(a, (8, 1))


# ---------------------------------------------------------------- device kernel
def _build_nc():
    import concourse.bass as bass
    import concourse.bacc as bacc
    import concourse.tile as tile
    import concourse.mybir as mybir

    F32 = mybir.dt.float32
    BF16 = mybir.dt.bfloat16
    I16 = mybir.dt.int16
    AF = mybir.ActivationFunctionType
    OP = mybir.AluOpType

    LR1, LR2 = _LR_RANGES["l1"], _LR_RANGES["l2"]
    nc = bacc.Bacc(None, target_bir_lowering=False, num_swdge_queues=4)

    # ---- inputs
    xoT = nc.dram_tensor("xoT", [128, SPC], BF16, kind="ExternalInput")
    xgT = nc.dram_tensor("xgT", [128, S], BF16, kind="ExternalInput")
    wl1 = nc.dram_tensor("wl1", [128, HC], BF16, kind="ExternalInput")
    wr1 = nc.dram_tensor("wr1", [128, HC], BF16, kind="ExternalInput")
    wl2 = nc.dram_tensor("wl2", [HC, HC], BF16, kind="ExternalInput")
    wr2 = nc.dram_tensor("wr2", [HC, HC], BF16, kind="ExternalInput")
    w3 = nc.dram_tensor("w3", [HC, 128], BF16, kind="ExternalInput")
    w4 = nc.dram_tensor("w4", [128, OUT_F], BF16, kind="ExternalInput")
    iavT1 = nc.dram_tensor("iavT1", [128, 2], F32, kind="ExternalInput")
    iavT2 = nc.dram_tensor("iavT2", [128, 2], F32, kind="ExternalInput")
    bT1 = nc.dram_tensor("bT1", [128, 2], F32, kind="ExternalInput")
    bT2 = nc.dram_tensor("bT2", [128, 2], F32, kind="ExternalInput")
    b3c = nc.dram_tensor("b3c", [128, 1], F32, kind="ExternalInput")
    b4f = nc.dram_tensor("b4f", [128, OUT_F], F32, kind="ExternalInput")
    idenBF = nc.dram_tensor("idenBF", [128, 128], BF16, kind="ExternalInput")
    epsc = nc.dram_tensor("epsc", [128, 1], F32, kind="ExternalInput")

    idxXLA = nc.dram_tensor("idxXLA", [NTILES, 128, TA * 8], I16,
                            kind="ExternalInput")
    idxXLB = nc.dram_tensor("idxXLB", [NTILES, 128, TB * 8], I16,
                            kind="ExternalInput")
    idxXLA1 = nc.dram_tensor("idxXLA1", [NTILES, 128, TA * 8], I16,
                             kind="ExternalInput")
    idxXLB1 = nc.dram_tensor("idxXLB1", [NTILES, 128, TB * 8], I16,
                             kind="ExternalInput")
    mkd = nc.dram_tensor("mkd", [NTILES, 128, NS * 128], BF16,
                         kind="ExternalInput")
    mkTd = nc.dram_tensor("mkTd", [NTILES, 128, TS * 128], BF16,
                          kind="ExternalInput")
    out_ext = nc.dram_tensor("out", [SPC, OUT_F], F32, kind="ExternalOutput")

    # ---- DRAM intermediates (a/b = gather table split at tile 24)
    RA = NTILES_A * 128           # own rows in table a (3072)
    RB = (NTILES - NTILES_A) * 128  # own rows in table b (3200)
    loc1 = nc.dram_tensor("loc1", [SPC, 2, HC], BF16)
    loc2 = nc.dram_tensor("loc2", [SPC, 2, HC], BF16)
    xl_all1a = nc.dram_tensor("xl_all1a", [HALFR, HC], BF16)
    xl_all1b = nc.dram_tensor("xl_all1b", [S - HALFR, HC], BF16)
    xl_own2a = nc.dram_tensor("xl_own2a", [RA, HC], BF16)
    xl_own2b = nc.dram_tensor("xl_own2b", [RB, HC], BF16)
    xl_all2a = nc.dram_tensor("xl_all2a", [HALFR, HC], BF16,
                              addr_space="Shared")
    xl_all2b = nc.dram_tensor("xl_all2b", [S - HALFR, HC], BF16,
                              addr_space="Shared")

    # per-chunk AllGather metadata
    ag_meta = []
    for c in range(len(AG_CH)):
        t0, nt, tab = AG_T0[c], AG_CH[c], AG_TAB[c]
        own_r0 = (t0 - (0 if tab == 0 else NTILES_A)) * 128
        all_r0 = AG_BASE[c] - (0 if tab == 0 else HALFR)
        ag_meta.append((t0 + nt - 1, tab, own_r0, all_r0, nt * 128))

    with tile.TileContext(nc) as tc:
        with (
            tc.tile_pool(name="const", bufs=1) as cpool,
            tc.tile_pool(name="tabw", bufs=2) as tabw,
            tc.tile_pool(name="ld2", bufs=2) as ld2,
            tc.tile_pool(name="gath", bufs=4) as gpool,
            tc.tile_pool(name="work", bufs=2) as wpool,
            tc.tile_pool(name="fin", bufs=2) as fpool,
            tc.tile_pool(name="fint", bufs=4) as ftpool,
            tc.tile_pool(name="psu", bufs=2, space="PSUM") as psu,
            tc.tile_pool(name="psx", bufs=2, space="PSUM") as psx,
            tc.tile_pool(name="psT", bufs=2, space="PSUM") as psT,
        ):
            # ---------- persistent constants in SBUF
            def load_const(t, shape, dt):
                tl = cpool.tile(shape, dt, tag=t.name, name=t.name + "_sb")
                nc.sync.dma_start(out=tl[:], in_=t[:])
                return tl

            wl1_sb = load_const(wl1, [128, HC], BF16)
            wr1_sb = load_const(wr1, [128, HC], BF16)
            w4_sb = load_const(w4, [128, OUT_F], BF16)
            iavT1_sb = load_const(iavT1, [128, 2], F32)
            iavT2_sb = load_const(iavT2, [128, 2], F32)
            bT1_sb = load_const(bT1, [128, 2], F32)
            bT2_sb = load_const(bT2, [128, 2], F32)
            b3c_sb = load_const(b3c, [128, 1], F32)
            b4f_sb = load_const(b4f, [128, OUT_F], F32)
            iden_sb = load_const(idenBF, [128, 128], BF16)
            epsc_sb = load_const(epsc, [128, 1], F32)


            def load_const2(t, cols, tag):
                tl = cpool.tile([128, 2, cols], BF16, tag=tag, name=tag + "_sb")
                nc.sync.dma_start(
                    out=tl[:], in_=t.rearrange("(a p) c -> p a c", p=128))
                return tl

            wl2_sb = load_const2(wl2, HC, "wl2x")
            wr2_sb = load_const2(wr2, HC, "wr2x")
            w3_sb = load_const2(w3, 128, "w3x")

            def own_slice(owna, ownb, t):
                if t < NTILES_A:
                    return owna[t * 128:(t + 1) * 128, :]
                tb = t - NTILES_A
                return ownb[tb * 128:(tb + 1) * 128, :]

            def all_slice(alla, allb, gt):
                if gt < HALFR // 128:
                    return alla[gt * 128:(gt + 1) * 128, :]
                gb_ = gt - HALFR // 128
                return allb[gb_ * 128:(gb_ + 1) * 128, :]

            def ag_fire(owns, alls, t):
                """Fire any AllGather chunk whose last tile is t."""
                for (lt, tab, own_r0, all_r0, nr) in ag_meta:
                    if lt != t:
                        continue
                    nc.gpsimd.collective_compute(
                        "AllGather", mybir.AluOpType.bypass,
                        replica_groups=[list(range(NCORES))],
                        ins=[owns[tab][own_r0:own_r0 + nr, :]],
                        outs=[alls[tab][all_r0:all_r0 + nr * NCORES, :]])

            # ---------- L1 tables, replicated: every core computes the FULL
            # xl1 gather table from x (no collective), plus its own loc1.
            def table_local_l1():
                for t in range(NTILES):
                    lt = tabw.tile([128, 128], BF16, tag="tablhs")
                    nc.sync.dma_start(out=lt[:],
                                      in_=xoT[:, t * 128:(t + 1) * 128])
                    ot = tabw.tile([128, 2, HC], BF16, tag="tabout")
                    for j, w_sb in ((0, wl1_sb), (1, wr1_sb)):
                        pst = psu.tile([128, 258], F32, tag="u")
                        nc.tensor.matmul(pst[:, 0:HC], lt[:], w_sb[:],
                                         start=True, stop=True)
                        if j == 0:
                            nc.vector.tensor_copy(ot[:, j, :], pst[:, 0:HC])
                        else:
                            nc.scalar.activation(ot[:, j, :], pst[:, 0:HC],
                                                 AF.Copy)
                    nc.scalar.dma_start(
                        out=loc1[t * 128:(t + 1) * 128, :, :], in_=ot[:])

            def table_full_l1():
                TG = 8
                for g0 in range(0, GTILES, TG):
                    lt = tabw.tile([128, TG * 128], BF16, tag="tabghs")
                    nc.sync.dma_start(out=lt[:],
                                      in_=xgT[:, g0 * 128:(g0 + TG) * 128])
                    og = tabw.tile([128, TG, HC], BF16, tag="tabgo")
                    for c in range(2):
                        pst = psx.tile([128, 4, HC], F32, tag="xrb")
                        for j in range(4):
                            nc.tensor.matmul(
                                pst[:, j, :],
                                lt[:, (c * 4 + j) * 128:(c * 4 + j + 1) * 128],
                                wl1_sb[:], start=True, stop=True)
                        if c == 0:
                            nc.vector.tensor_copy(og[:, 0:4, :], pst[:])
                        else:
                            nc.scalar.activation(og[:, 4:8, :], pst[:],
                                                 AF.Copy)
                    # groups of 8 tiles never straddle the A/B split;
                    # lane-major layout: row = p*ntt + t (4 KB contiguous
                    # per partition per group)
                    if g0 < HALFR // 128:
                        dst_t, t0g, ntt = xl_all1a, g0, HALFR // 128
                    else:
                        dst_t, t0g, ntt = (xl_all1b, g0 - HALFR // 128,
                                           (S - HALFR) // 128)
                    nc.sync.dma_start(
                        out=dst_t.rearrange("(p tt) c -> p tt c", p=128)[
                            :, t0g:t0g + TG, :],
                        in_=og[:])

            # ---------- edge phase (one conv layer), software-pipelined
            def conv_layer(xl_ta, xl_tb, ixta, ixtb, loc_tab, qf,
                           fin_pre_cb, fin_post_cb, ag_cb=None):
                n_batches = NTILES // GB + (1 if NTILES % GB else 0)
                st = {}

                def nb_of(bi):
                    return min(bi * GB + GB, NTILES) - bi * GB

                def stage_gather(bi):
                    t0, nb = bi * GB, nb_of(bi)
                    s = st.setdefault(bi, {})
                    ixa = ld2.tile([128, GB, TA * 8], I16, tag="ixa")
                    nc.sync.dma_start(
                        out=ixa[:, 0:nb],
                        in_=ixta[t0:t0 + nb].rearrange("t p c -> p t c"))
                    ixb = ld2.tile([128, GB, TB * 8], I16, tag="ixb")
                    nc.sync.dma_start(
                        out=ixb[:, 0:nb],
                        in_=ixtb[t0:t0 + nb].rearrange("t p c -> p t c"))
                    s["gA"] = gpool.tile([128, GB * TA, HC], BF16, tag="gA", name="gA_sb")
                    s["gB"] = gpool.tile([128, GB * TB, HC], BF16, tag="gB", name="gB_sb")
                    nsa = nb * TA
                    ixa_f = ixa[:, 0:nb].rearrange("p t c -> p (t c)")
                    ixb_f = ixb[:, 0:nb].rearrange("p t c -> p (t c)")
                    for g, ixf, tab, q0 in ((s["gA"], ixa_f, xl_ta, 0),
                                            (s["gB"], ixb_f, xl_tb, 2)):
                        h = nsa // 2
                        nc.gpsimd.dma_gather(
                            out_ap=g[:, 0:h, :], in_ap=tab[:, :],
                            idxs_ap=ixf[:, 0:h * 8],
                            num_idxs=h * 128, num_idxs_reg=h * 128,
                            elem_size=HC, single_packet=False, queue_num=q0)
                        nc.gpsimd.dma_gather(
                            out_ap=g[:, h:nsa, :], in_ap=tab[:, :],
                            idxs_ap=ixf[:, h * 8:nsa * 8],
                            num_idxs=(nsa - h) * 128,
                            num_idxs_reg=(nsa - h) * 128,
                            elem_size=HC, single_packet=False,
                            queue_num=q0 + 1)

                def stage_load_mkT(bi):
                    t0, nb = bi * GB, nb_of(bi)
                    s = st.setdefault(bi, {})
                    s["mkT"] = ld2.tile([128, GB, TS * 128], BF16, tag="mkT", name="mkT_sb")
                    nc.sync.dma_start(
                        out=s["mkT"][:, 0:nb],
                        in_=mkTd[t0:t0 + nb].rearrange("t p c -> p t c"))

                def stage_load_near(bi):
                    t0, nb = bi * GB, nb_of(bi)
                    s = st.setdefault(bi, {})
                    s["sxb"] = ld2.tile([128, GB, 2, HC], BF16, tag="sxb", name="sxb_sb")
                    nc.sync.dma_start(
                        out=s["sxb"][:, 0:nb],
                        in_=loc_tab[t0 * 128:(t0 + nb) * 128].rearrange(
                            "(a p) b c -> p a b c", p=128))
                    s["mk"] = ld2.tile([128, GB, NS * 128], BF16, tag="mk", name="mk_sb")
                    nc.sync.dma_start(
                        out=s["mk"][:, 0:nb],
                        in_=mkd[t0:t0 + nb].rearrange("t p c -> p t c"))

                def stage_u(bi):
                    nb = nb_of(bi)
                    s = st[bi]
                    gA, gB, sxb, mkT = s["gA"], s["gB"], s["sxb"], s["mkT"]
                    work = wpool.tile([128, GB * NS, 258], BF16, tag="work")
                    s["work"] = work
                    w4d = work[:, :, 0:HC].rearrange(
                        "p (t s) c -> p t s c", s=NS)
                    s["w4d"] = w4d
                    # u = xl[src] + xr[dst]: xr via one-hot matmul, xl via
                    # identity-matmul accumulate, ACT drains chunks to bf16.
                    for ti in range(nb):
                        for s0 in range(0, TS, 4):
                            s1 = min(s0 + 4, TS)
                            xrb = psx.tile([128, 4, HC], F32, tag="xrb")
                            for si in range(s0, s1):
                                nc.tensor.matmul(
                                    xrb[:, si - s0, :],
                                    mkT[:, ti, si * 128:(si + 1) * 128],
                                    sxb[:, ti, 1, :], start=True, stop=False)
                                g, gofs = (gA, 0) if si < TA else (gB, TA)
                                nc.tensor.matmul(
                                    xrb[:, si - s0, :], iden_sb[:],
                                    g[:, ti * TA + si - gofs, :],
                                    start=False, stop=True)
                            nc.scalar.activation(
                                w4d[:, ti, s0:s1, :], xrb[:, 0:s1 - s0, :],
                                AF.Prelu, alpha=SLOPE)
                    # self subtile: loc xl + xr, then leaky (max form —
                    # tables are |att|-scaled so every column is max-form)
                    nc.vector.tensor_tensor(
                        out=w4d[:, 0:nb, TS, :],
                        in0=sxb[:, 0:nb, 0, :], in1=sxb[:, 0:nb, 1, :],
                        op=OP.add)
                    nc.vector.scalar_tensor_tensor(
                        out=w4d[:, 0:nb, TS, :], in0=w4d[:, 0:nb, TS, :],
                        scalar=SLOPE, in1=w4d[:, 0:nb, TS, :],
                        op0=OP.mult, op1=OP.max)

                def stage_score(bi):
                    nb = nb_of(bi)
                    s = st[bi]
                    qs, flips = qf
                    work = s["work"]
                    wf = work[:, 0:nb * NS]
                    wh = wf[:, :, 0:HC].rearrange("p s (h c) -> p s h c", h=2)
                    # flip the minority-sign members of each head's single
                    # mixed quad, then the fold tree is sign-pure per column
                    for h in range(2):
                        for c in flips[h]:
                            nc.vector.tensor_scalar(
                                out=wh[:, :, h, c:c + 1],
                                in0=wh[:, :, h, c:c + 1],
                                scalar1=-1.0, scalar2=None, op0=OP.mult)
                    nc.vector.tensor_tensor(
                        out=wh[:, :, :, 0:64], in0=wh[:, :, :, 0:64],
                        in1=wh[:, :, :, 64:128], op=OP.add)
                    nc.vector.tensor_tensor(
                        out=wh[:, :, :, 0:32], in0=wh[:, :, :, 0:32],
                        in1=wh[:, :, :, 32:64], op=OP.add)
                    # score = sum(pos finals) - sum(neg finals) per head
                    sc = wpool.tile([128, GB * NS, 2], F32, tag="sc")
                    sn = wpool.tile([128, GB * NS, 2], F32, tag="sn")
                    for h in range(2):
                        q = qs[h]
                        nc.vector.tensor_reduce(
                            out=sc[:, 0:nb * NS, h:h + 1].rearrange(
                                "p s h -> p s h ()"),
                            in_=wh[:, :, h:h + 1, 0:q],
                            axis=mybir.AxisListType.X, op=OP.add)
                        nc.vector.tensor_reduce(
                            out=sn[:, 0:nb * NS, h:h + 1].rearrange(
                                "p s h -> p s h ()"),
                            in_=wh[:, :, h:h + 1, q:32],
                            axis=mybir.AxisListType.X, op=OP.add)
                    nc.vector.tensor_tensor(
                        out=sc[:, 0:nb * NS], in0=sc[:, 0:nb * NS],
                        in1=sn[:, 0:nb * NS], op=OP.subtract)
                    af = wpool.tile([128, GB * NS, 2], F32, tag="af")
                    s["af"] = af
                    nc.scalar.activation(af[:, 0:nb * NS], sc[:, 0:nb * NS],
                                         AF.Exp)
                    nc.scalar.activation(work[:, 0:nb * NS, HC:HC + 2],
                                         af[:, 0:nb * NS], AF.Copy)

                def stage_y(bi):
                    nb = nb_of(bi)
                    s = st[bi]
                    gA, gB, sxb, w4d = s["gA"], s["gB"], s["sxb"], s["w4d"]
                    af = s["af"]
                    for ti in range(nb):
                        for h, eng in ((0, nc.vector), (1, nc.gpsimd)):
                            ab = af[:, ti * NS:(ti + 1) * NS,
                                    h:h + 1].broadcast_to([128, NS, 128])
                            cl, ch = h * 128, h * 128 + 128
                            eng.tensor_tensor(
                                out=w4d[:, ti, 0:TA, cl:ch],
                                in0=gA[:, ti * TA:(ti + 1) * TA, cl:ch],
                                in1=ab[:, 0:TA], op=OP.mult)
                            eng.tensor_tensor(
                                out=w4d[:, ti, TA:TS, cl:ch],
                                in0=gB[:, ti * TB:(ti + 1) * TB, cl:ch],
                                in1=ab[:, TA:TS], op=OP.mult)
                            eng.tensor_tensor(
                                out=w4d[:, ti, TS, cl:ch],
                                in0=sxb[:, ti, 0, cl:ch],
                                in1=ab[:, TS], op=OP.mult)

                def stage_scatter(bi):
                    t0, nb = bi * GB, nb_of(bi)
                    s = st[bi]
                    mk, work = s["mk"], s["work"]
                    pres = []
                    for ti in range(nb):
                        u_ps = psu.tile([128, 258], F32, tag="u")
                        for si in range(NS):
                            nc.tensor.matmul(
                                u_ps[:], mk[:, ti, si * 128:(si + 1) * 128],
                                work[:, ti * NS + si, :],
                                start=(si == 0), stop=(si == NS - 1))
                        pres.append(fin_pre_cb(u_ps))
                    s["pres"] = pres

                def stage_finpost(bi):
                    t0, nb = bi * GB, nb_of(bi)
                    pres = st[bi]["pres"]
                    for ti in range(nb):
                        fin_post_cb(t0 + ti, pres[ti])
                        if ag_cb is not None:
                            ag_cb(t0 + ti)
                    del st[bi]

                # ---- pipelined schedule: gathers dispatched 3 batches
                # ahead, masks/sxb loaded 1-2 ahead, fin writes on ACT-DMA
                for bj in range(min(3, n_batches)):
                    stage_gather(bj)
                stage_load_mkT(0)
                if n_batches > 1:
                    stage_load_mkT(1)
                stage_load_near(0)
                stage_u(0)
                for bi in range(n_batches):
                    if bi + 1 < n_batches:
                        stage_load_near(bi + 1)
                    stage_score(bi)
                    if bi > 0:
                        stage_finpost(bi - 1)
                    if bi + 3 < n_batches:
                        stage_gather(bi + 3)
                    if bi + 2 < n_batches:
                        stage_load_mkT(bi + 2)
                    if bi + 1 < n_batches:
                        stage_u(bi + 1)
                    stage_y(bi)
                    stage_scatter(bi)
                stage_finpost(n_batches - 1)

            # ---------- finalize, split so PE never stalls behind the
            # ACT normalization chain: fin_pre (ACT/DVE, frees u_ps) then
            # fin_post (PE transposes + dense tail) after the next scatters.
            def fin_pre(u_ps, iavT_sb, bT_sb):
                dcol = fpool.tile([128, 2], F32, tag="dcol")
                nc.vector.tensor_scalar(
                    out=dcol[:], in0=u_ps[:, HC:HC + 2],
                    scalar1=epsc_sb[:, 0:1], scalar2=None, op0=OP.add)
                rcol = fpool.tile([128, 2], F32, tag="rcol")
                nc.vector.reciprocal(rcol[:], dcol[:])
                t1 = ftpool.tile([128, 2, 128], BF16, tag="t1")
                for h in range(2):
                    nc.vector.tensor_scalar(
                        out=t1[:, h, :], in0=u_ps[:, h * 128:(h + 1) * 128],
                        scalar1=rcol[:, h:h + 1], scalar2=None, op0=OP.mult)
                return t1

            def fin_tc(t1, iavT_sb, bT_sb):
                pt = psT.tile([128, 2, 128], BF16, tag="fps")
                for h in range(2):
                    nc.tensor.transpose(pt[:, h, :], t1[:, h, :], iden_sb[:])
                cts = []
                for h in range(2):
                    ct = fpool.tile([128, 128], BF16, tag=f"ct{h}")
                    nc.scalar.activation(ct[:], pt[:, h, :], AF.Relu,
                                         scale=iavT_sb[:, h:h + 1],
                                         bias=bT_sb[:, h:h + 1])
                    cts.append(ct)
                return cts

            def fin1_pre(u_ps):
                return fin_pre(u_ps, iavT1_sb, bT1_sb)

            def fin1_post(t, t1):
                cts = fin_tc(t1, iavT1_sb, bT1_sb)
                ot2 = fpool.tile([128, 2, HC], BF16, tag="ot2")
                for j, w2_sb in ((0, wl2_sb), (1, wr2_sb)):
                    pst = psu.tile([128, 258], F32, tag="u")
                    nc.tensor.matmul(pst[:, 0:HC], cts[0][:], w2_sb[:, 0, :],
                                     start=True, stop=False)
                    nc.tensor.matmul(pst[:, 0:HC], cts[1][:], w2_sb[:, 1, :],
                                     start=False, stop=True)
                    nc.scalar.activation(ot2[:, j, :], pst[:, 0:HC], AF.Copy)
                nc.scalar.dma_start(
                    out=loc2[t * 128:(t + 1) * 128, :, :], in_=ot2[:])
                nc.scalar.dma_start(
                    out=own_slice(xl_own2a, xl_own2b, t), in_=ot2[:, 0, :])

            def fin2_pre(u_ps):
                return fin_pre(u_ps, iavT2_sb, bT2_sb)

            def fin2_post(t, t1):
                cts = fin_tc(t1, iavT2_sb, bT2_sb)
                zt_ps = psu.tile([128, 258], F32, tag="u")
                nc.tensor.matmul(zt_ps[:, 0:128], w3_sb[:, 0, :], cts[0][:],
                                 start=True, stop=False)
                nc.tensor.matmul(zt_ps[:, 0:128], w3_sb[:, 1, :], cts[1][:],
                                 start=False, stop=True)
                zt_sb = fpool.tile([128, 128], BF16, tag="ztsb")
                nc.scalar.activation(zt_sb[:], zt_ps[:, 0:128], AF.Identity,
                                     bias=b3c_sb[:, 0:1])
                o_ps = psu.tile([128, 258], F32, tag="u")
                nc.tensor.matmul(o_ps[:, 0:OUT_F], zt_sb[:], w4_sb[:],
                                 start=True, stop=True)
                o_pre = fpool.tile([128, OUT_F], F32, tag="opre")
                nc.vector.scalar_tensor_tensor(
                    out=o_pre[:], in0=o_ps[:, 0:OUT_F], scalar=1.0,
                    in1=b4f_sb[:], op0=OP.mult, op1=OP.add)
                o_sb = fpool.tile([128, OUT_F], F32, tag="osb")
                nc.scalar.activation(o_sb[:], o_pre[:], AF.Sigmoid)
                nc.scalar.dma_start(out=out_ext[t * 128:(t + 1) * 128, :],
                                    in_=o_sb[:])

            # ================= phase schedule =================
            import os as _os
            _upto = int(_os.environ.get("KPHASES", "9"))

            table_local_l1()
            table_full_l1()
            if _upto >= 2:
                ag2 = lambda t: ag_fire((xl_own2a, xl_own2b),
                                        (xl_all2a, xl_all2b), t)
                conv_layer(xl_all1a, xl_all1b, idxXLA1, idxXLB1, loc1,
                           LR1, fin1_pre, fin1_post, ag_cb=ag2)
            if _upto >= 4:
                conv_layer(xl_all2a, xl_all2b, idxXLA, idxXLB, loc2,
                           LR2, fin2_pre, fin2_post)
            else:
                zt = fpool.tile([128, OUT_F], F32, tag="osb")
                nc.vector.memset(zt[:], 0.0)
                for t in range(NTILES):
                    nc.sync.dma_start(out=out_ext[t * 128:(t + 1) * 128, :],
                                      in_=zt[:])

    nc.compile()
    return nc


# ---------------------------------------------------------------- entry point
def kernel(**inputs):
    from concourse import bass_utils

    src = np.asarray(inputs["edge_index"][0], np.int64)
    dst = np.asarray(inputs["edge_index"][1], np.int64)
    x = np.asarray(inputs["x"], np.float32)

    pack = _pack_graph(src, dst)
    nos = pack["node_of_slot"]
    valid = nos >= 0
    x_slot = np.zeros((S, IN_F), np.float32)
    x_slot[valid] = x[nos[valid]]
    sog = pack["slot_of_grow"]
    x_grow = np.zeros((S, IN_F), np.float32)
    gv = sog >= 0
    x_grow[gv] = x_slot[sog[gv]]

    def bf(a):
        return np.ascontiguousarray(np.asarray(a, np.float32)).astype(BF)

    # --- per-head column permutation: sign-pure quads so the fold tree
    # (c, c+32, c+64, c+96 -> final col c) only combines same-sign columns;
    # score = reduce(pos finals) - reduce(neg finals). At most one mixed
    # quad per head; its minority-sign members get flipped on-device.
    def prep_layer(att):
        att = np.asarray(att, np.float32).reshape(2, 128)
        perm = np.zeros(HC, np.int64)
        qs, flips = [], []
        for h in range(2):
            a = att[h]
            pos = list(np.where(a > 0)[0])
            neg = list(np.where(a <= 0)[0])
            p = len(pos)
            np_q = p // 4
            mixed = 1 if p % 4 else 0
            colassign = np.empty((4, 32), np.int64)
            pi = ni = 0
            fl = []
            for j in range(32):
                if j < np_q:
                    for m in range(4):
                        colassign[m, j] = pos[pi]; pi += 1
                elif mixed and j == np_q:
                    for m in range(4):
                        if pi < p:
                            colassign[m, j] = pos[pi]; pi += 1
                        else:
                            colassign[m, j] = neg[ni]; ni += 1
                            fl.append(m * 32 + j)
                else:
                    for m in range(4):
                        colassign[m, j] = neg[ni]; ni += 1
            for m in range(4):
                for j in range(32):
                    perm[h * 128 + m * 32 + j] = h * 128 + colassign[m, j]
            q = np_q + mixed
            assert 0 < q < 32, f"degenerate sign split p={p}"
            qs.append(q)
            flips.append(fl)
        att_p = att.reshape(HC)[perm]
        att_p = np.where(np.abs(att_p) < 1e-30, 1e-30, att_p)
        att_p = np.abs(att_p)
        return perm, att_p, (qs, flips)

    perm1, att1p, LR1 = prep_layer(inputs["att1"])
    perm2, att2p, LR2 = prep_layer(inputs["att2"])
    _LR_RANGES["l1"] = LR1
    _LR_RANGES["l2"] = LR2

    Wl1p = np.asarray(inputs["Wl1"], np.float32)[:, perm1] * att1p[None, :]
    Wr1p = np.asarray(inputs["Wr1"], np.float32)[:, perm1] * att1p[None, :]
    Wl2p = (np.asarray(inputs["Wl2"], np.float32)[perm1][:, perm2]
            * att2p[None, :])
    Wr2p = (np.asarray(inputs["Wr2"], np.float32)[perm1][:, perm2]
            * att2p[None, :])
    W3p = np.asarray(inputs["W3"], np.float32)[perm2]
    b1p = np.asarray(inputs["b1"], np.float32)[perm1]
    b2p = np.asarray(inputs["b2"], np.float32)[perm2]

    common = {
        "wl1": bf(Wl1p), "wr1": bf(Wr1p),
        "wl2": bf(Wl2p), "wr2": bf(Wr2p),
        "w3": bf(W3p), "w4": bf(inputs["W4"]),
        "iavT1": np.ascontiguousarray(
            (1.0 / att1p).reshape(2, 128).T.astype(np.float32)),
        "iavT2": np.ascontiguousarray(
            (1.0 / att2p).reshape(2, 128).T.astype(np.float32)),
        "bT1": np.ascontiguousarray(b1p.reshape(2, 128).T.astype(np.float32)),
        "bT2": np.ascontiguousarray(b2p.reshape(2, 128).T.astype(np.float32)),
        "b3c": np.asarray(inputs["b3"], np.float32).reshape(128, 1),
        "b4f": np.tile(np.asarray(inputs["b4"], np.float32)[None, :], (128, 1)),
        "idenBF": np.eye(128, dtype=np.float32).astype(BF),
        "epsc": np.full((128, 1), 1e-16, np.float32),

        "xgT": np.ascontiguousarray(x_grow.T).astype(BF),
    }

    in_maps = []
    for k in range(NCORES):
        m = dict(common)
        m["xoT"] = np.ascontiguousarray(
            x_slot[k * SPC:(k + 1) * SPC].T).astype(BF)
        ixla = np.empty((NTILES, 128, TA * 8), np.int16)
        ixlb = np.empty((NTILES, 128, TB * 8), np.int16)
        ixla1 = np.empty((NTILES, 128, TA * 8), np.int16)
        ixlb1 = np.empty((NTILES, 128, TB * 8), np.int16)
        for t in range(NTILES):
            ixla[t] = _wrap_idx(pack["idxXL"][k, t, :TA * 128])
            ixlb[t] = _wrap_idx(pack["idxXL"][k, t, TA * 128:])
            ixla1[t] = _wrap_idx(pack["idxXL1"][k, t, :TA * 128])
            ixlb1[t] = _wrap_idx(pack["idxXL1"][k, t, TA * 128:])
        m["idxXLA"] = ixla
        m["idxXLB"] = ixlb
        m["idxXLA1"] = ixla1
        m["idxXLB1"] = ixlb1
        m["mkd"] = pack["mk"][k].astype(BF)
        m["mkTd"] = pack["mkT"][k].astype(BF)
        in_maps.append(m)

    if "nc" not in _NC_CACHE:
        _NC_CACHE["nc"] = _build_nc()
    nc = _NC_CACHE["nc"]

    res = bass_utils.run_bass_kernel_spmd(nc, in_maps,
                                          core_ids=list(range(NCORES)),
                                          **_RUN_OPTS)
    _LAST_RESULTS["res"] = res
    out_slots = np.concatenate([res.results[k]["out"] for k in range(NCORES)], 0)
    return out_slots[pack["slot_of_node"]].astype(np.float32)


# revision 36
# speedup vs baseline: 1.1016x; 1.1016x over previous
"""GATv2 (2-layer, 2-head) Trainium2 kernel, 8-core SPMD — v5.

vs v4: layer-1 xl table computed redundantly on every core (no L1
AllGather, fast startup), conv batches software-pipelined (u-phase of
batch b+1 issues before score/y/scatter of batch b so PE never waits
behind the DVE/ACT chain), DMA loads prefetch 2 batches ahead, leaky
back to single scalar_tensor_tensor, y fully on DVE.
"""
import sys

sys.path.insert(0, "/opt/trn_rl_repo")

import numpy as np
import ml_dtypes

BF = ml_dtypes.bfloat16

# ---- static layout constants (match reference problem sizes) ----
N = 50000
NCORES = 8
LANES = 128
NTILES = 49
SPC = NTILES * LANES          # 6272 slots per core
S = NCORES * SPC              # 50176 total slots
GTILES = S // 128             # 392 gather-table tiles
TA = 7                        # table-A gather subtiles per dst-tile
TB = 7
TS = TA + TB                  # random-edge subtiles (self subtile is extra)
NS = TS + 1                   # subtiles per tile incl self
GB = 3                        # dst-tiles per gather batch
IN_F = 128
HC = 256                      # H*C
OUT_F = 40
SLOPE = 0.2
# AllGather chunking (layer 2 only): 5 tile groups (sum = NTILES).
# Groups 0,1 make up gather table A, groups 2,3,4 table B.
AG_CH = (16, 8, 12, 8, 5)
AG_T0 = (0, 16, 24, 36, 44)
AG_TAB = (0, 0, 1, 1, 1)
NTILES_A = 24                 # tiles in table A
HALFR = NTILES_A * LANES * NCORES   # 24576 rows in table A
_b = [0] * len(AG_CH)
_acc = [0, 0]
for _c in range(len(AG_CH)):
    _b[_c] = (0 if AG_TAB[_c] == 0 else HALFR) + _acc[AG_TAB[_c]]
    _acc[AG_TAB[_c]] += AG_CH[_c] * LANES * NCORES
AG_BASE = tuple(_b)

_NC_CACHE = {}
_RUN_OPTS = {}
_LAST_RESULTS = {}
_LR_RANGES = {}


# ---------------------------------------------------------------- host prep
def _pack_graph(src, dst):
    deg = np.bincount(dst, minlength=N)

    is_self = src == dst
    self_eids = np.full(N, -1, np.int64)
    sids = np.where(is_self)[0]
    self_eids[src[sids]] = sids
    rand_mask = np.ones(len(src), bool)
    rand_mask[self_eids[self_eids >= 0]] = False

    nodes_per_core = (N + NCORES - 1) // NCORES
    order = np.argsort(-deg, kind="stable")
    core_edges = np.zeros(NCORES, np.int64)
    core_nodes = np.zeros(NCORES, np.int64)
    core_of_node = np.full(N, -1, np.int32)
    for v in order:
        k = np.argmin(np.where(core_nodes < nodes_per_core, core_edges, 1 << 60))
        core_of_node[v] = k
        core_edges[k] += deg[v]
        core_nodes[k] += 1

    rsrc, rdst = src[rand_mask], dst[rand_mask]

    # --- chunk-group assignment per core (before tile packing): deal nodes
    # round-robin by out-degree so the gather-table halves stay balanced.
    NG = len(AG_CH)
    odeg = np.bincount(rsrc, minlength=N)
    group_of_node = np.full(N, -1, np.int8)
    gcap = [c * LANES for c in AG_CH]
    for k in range(NCORES):
        vs = np.where(core_of_node == k)[0]
        vs = vs[np.argsort(-odeg[vs], kind="stable")]
        cnt = [0] * NG
        gi = 0
        for v in vs:
            while cnt[gi % NG] >= gcap[gi % NG]:
                gi += 1
            group_of_node[v] = gi % NG
            cnt[gi % NG] += 1
            gi += 1
    eh_node = np.asarray(AG_TAB, np.int8)[group_of_node]

    dA = np.bincount(rdst[eh_node[rsrc] == 0], minlength=N)
    dB = np.bincount(rdst[eh_node[rsrc] == 1], minlength=N)
    capA, capB = TA * LANES, TB * LANES

    tile_of_node = np.full(N, -1, np.int32)
    lane_of_node = np.full(N, -1, np.int32)
    for k in range(NCORES):
        for g in range(NG):
            vs = np.where((core_of_node == k) & (group_of_node == g))[0]
            vs = vs[np.argsort(-(dA[vs] + dB[vs]), kind="stable")]
            nv = len(vs)
            ntg = AG_CH[g]
            tile = np.empty(nv, np.int64)
            for i in range(nv):
                r, c = divmod(i, ntg)
                tile[i] = c if r % 2 == 0 else ntg - 1 - c
            loadA = np.bincount(tile, weights=dA[vs],
                                minlength=ntg).astype(np.int64)
            loadB = np.bincount(tile, weights=dB[vs],
                                minlength=ntg).astype(np.int64)
            it = 0
            while (loadA.max() > capA or loadB.max() > capB) and it < 100000:
                it += 1
                t_bad = int(np.argmax(np.maximum(loadA - capA, loadB - capB)))
                overA = loadA[t_bad] - capA >= loadB[t_bad] - capB
                t_good = int(np.argmin(loadA + loadB))
                in_bad = np.where(tile == t_bad)[0]
                in_good = np.where(tile == t_good)[0]
                d_bad = dA[vs[in_bad]] if overA else dB[vs[in_bad]]
                ib = in_bad[np.argmax(d_bad)]
                ig = in_good[np.argmin(dA[vs[in_good]] + dB[vs[in_good]])]
                for i, frm, to in ((ib, t_bad, t_good), (ig, t_good, t_bad)):
                    v = vs[i]
                    tile[i] = to
                    loadA[frm] -= dA[v]; loadA[to] += dA[v]
                    loadB[frm] -= dB[v]; loadB[to] += dB[v]
            if loadA.max() > capA or loadB.max() > capB:
                raise RuntimeError("edge packing failed; need bigger TA/TB")
            tile_of_node[vs] = AG_T0[g] + tile
            for t in range(ntg):
                nodes_t = vs[tile == t]
                lane_of_node[nodes_t] = np.arange(len(nodes_t))

    slot_of_node = (core_of_node.astype(np.int64) * SPC
                    + tile_of_node * LANES + lane_of_node)
    node_of_slot = np.full(S, -1, np.int64)
    node_of_slot[slot_of_node] = np.arange(N)

    # chunk-major gather-table row of each node
    g_arr = group_of_node.astype(np.int64)
    base = np.asarray(AG_BASE, np.int64)[g_arr]
    t0 = np.asarray(AG_T0, np.int64)[g_arr]
    chw = np.asarray(AG_CH, np.int64)[g_arr]
    grow_of_node = (base + core_of_node * chw * LANES
                    + (tile_of_node - t0) * LANES + lane_of_node)

    srcrow = grow_of_node[rsrc]
    dstslot = slot_of_node[rdst]
    dst_core = (dstslot // SPC).astype(np.int32)
    dst_tile = ((dstslot % SPC) // LANES).astype(np.int32)
    dst_lane = (dstslot % LANES).astype(np.int32)
    eh = (srcrow >= HALFR).astype(np.int8)

    idxXL = np.zeros((NCORES, NTILES, TS * 128), np.int16)
    idxXL1 = np.zeros((NCORES, NTILES, TS * 128), np.int16)

    key = (dst_core.astype(np.int64) * NTILES + dst_tile) * 2 + eh
    es = np.argsort(key, kind="stable")
    ksrc = srcrow[es]; kdl = dst_lane[es]
    kc = dst_core[es]; kt = dst_tile[es]; kh = eh[es]
    gkey = key[es]
    start = np.zeros(len(es), bool)
    start[0] = True
    start[1:] = gkey[1:] != gkey[:-1]
    gs = np.where(start, np.arange(len(es)), 0)
    gidx = np.arange(len(es)) - np.maximum.accumulate(gs)
    off = np.where(kh == 0, 0, TA * 128) + gidx
    tabrow = np.where(kh == 0, ksrc, ksrc - HALFR).astype(np.int64)
    idxXL[kc, kt, off] = tabrow.astype(np.int16)
    # conv1 gathers read the lane-major replicated L1 table:
    # row' = lane*(tiles in table) + tile
    ntt = np.where(kh == 0, HALFR // 128, (S - HALFR) // 128)
    row1 = (tabrow % 128) * ntt + tabrow // 128
    idxXL1[kc, kt, off] = row1.astype(np.int16)

    # one-hot masks: mk [e-lane -> dst-lane] per subtile (incl self at TS),
    # mkT [dst-lane -> e-lane] per random subtile.
    ksi = (off // 128).astype(np.int64)
    kel = (off % 128).astype(np.int64)
    mk = np.zeros((NCORES, NTILES, 128, NS * 128), np.float32)
    mkT = np.zeros((NCORES, NTILES, 128, TS * 128), np.float32)
    mk[kc, kt, kel, ksi * 128 + kdl] = 1.0
    mkT[kc, kt, kdl, ksi * 128 + kel] = 1.0
    vsel = np.where(self_eids >= 0)[0]
    ln = lane_of_node[vsel].astype(np.int64)
    mk[core_of_node[vsel], tile_of_node[vsel], ln, TS * 128 + ln] = 1.0

    # grow-order slot map (for the replicated L1 table build)
    slot_of_grow = np.full(S, -1, np.int64)
    slot_of_grow[grow_of_node[np.arange(N)]] = slot_of_node

    return dict(slot_of_node=slot_of_node, node_of_slot=node_of_slot,
                idxXL=idxXL, idxXL1=idxXL1, mk=mk, mkT=mkT,
                slot_of_grow=slot_of_grow)


def _wrap_idx(idx):
    """[n] -> [128, n//16] wrapped (j at partition j%16, col j//16) + replicated."""
    n = idx.shape[0]
    a = idx.reshape(n // 16, 16).T.astype(np.int16)
    return np.tile(a, (8, 1))


# ---------------------------------------------------------------- device kernel
def _build_nc():
    import concourse.bass as bass
    import concourse.bacc as bacc
    import concourse.tile as tile
    import concourse.mybir as mybir

    F32 = mybir.dt.float32
    BF16 = mybir.dt.bfloat16
    I16 = mybir.dt.int16
    AF = mybir.ActivationFunctionType
    OP = mybir.AluOpType

    LR1, LR2 = _LR_RANGES["l1"], _LR_RANGES["l2"]
    nc = bacc.Bacc(None, target_bir_lowering=False, num_swdge_queues=4)

    # ---- inputs
    xoT = nc.dram_tensor("xoT", [128, SPC], BF16, kind="ExternalInput")
    xgT = nc.dram_tensor("xgT", [128, S], BF16, kind="ExternalInput")
    wl1 = nc.dram_tensor("wl1", [128, HC], BF16, kind="ExternalInput")
    wr1 = nc.dram_tensor("wr1", [128, HC], BF16, kind="ExternalInput")
    wl2 = nc.dram_tensor("wl2", [HC, HC], BF16, kind="ExternalInput")
    wr2 = nc.dram_tensor("wr2", [HC, HC], BF16, kind="ExternalInput")
    w3 = nc.dram_tensor("w3", [HC, 128], BF16, kind="ExternalInput")
    w4 = nc.dram_tensor("w4", [128, OUT_F], BF16, kind="ExternalInput")
    iavT1 = nc.dram_tensor("iavT1", [128, 2], F32, kind="ExternalInput")
    iavT2 = nc.dram_tensor("iavT2", [128, 2], F32, kind="ExternalInput")
    bT1 = nc.dram_tensor("bT1", [128, 2], F32, kind="ExternalInput")
    bT2 = nc.dram_tensor("bT2", [128, 2], F32, kind="ExternalInput")
    b3c = nc.dram_tensor("b3c", [128, 1], F32, kind="ExternalInput")
    b4f = nc.dram_tensor("b4f", [128, OUT_F], F32, kind="ExternalInput")
    idenBF = nc.dram_tensor("idenBF", [128, 128], BF16, kind="ExternalInput")
    epsc = nc.dram_tensor("epsc", [128, 1], F32, kind="ExternalInput")

    idxXLA = nc.dram_tensor("idxXLA", [NTILES, 128, TA * 8], I16,
                            kind="ExternalInput")
    idxXLB = nc.dram_tensor("idxXLB", [NTILES, 128, TB * 8], I16,
                            kind="ExternalInput")
    idxXLA1 = nc.dram_tensor("idxXLA1", [NTILES, 128, TA * 8], I16,
                             kind="ExternalInput")
    idxXLB1 = nc.dram_tensor("idxXLB1", [NTILES, 128, TB * 8], I16,
                             kind="ExternalInput")
    mkd = nc.dram_tensor("mkd", [NTILES, 128, NS * 128], BF16,
                         kind="ExternalInput")
    mkTd = nc.dram_tensor("mkTd", [NTILES, 128, TS * 128], BF16,
                          kind="ExternalInput")
    out_ext = nc.dram_tensor("out", [SPC, OUT_F], F32, kind="ExternalOutput")

    # ---- DRAM intermediates (a/b = gather table split at tile 24)
    RA = NTILES_A * 128           # own rows in table a (3072)
    RB = (NTILES - NTILES_A) * 128  # own rows in table b (3200)
    loc1 = nc.dram_tensor("loc1", [SPC, 2, HC], BF16)
    loc2 = nc.dram_tensor("loc2", [SPC, 2, HC], BF16)
    xl_all1a = nc.dram_tensor("xl_all1a", [HALFR, HC], BF16)
    xl_all1b = nc.dram_tensor("xl_all1b", [S - HALFR, HC], BF16)
    xl_own2a = nc.dram_tensor("xl_own2a", [RA, HC], BF16)
    xl_own2b = nc.dram_tensor("xl_own2b", [RB, HC], BF16)
    xl_all2a = nc.dram_tensor("xl_all2a", [HALFR, HC], BF16,
                              addr_space="Shared")
    xl_all2b = nc.dram_tensor("xl_all2b", [S - HALFR, HC], BF16,
                              addr_space="Shared")

    # per-chunk AllGather metadata
    ag_meta = []
    for c in range(len(AG_CH)):
        t0, nt, tab = AG_T0[c], AG_CH[c], AG_TAB[c]
        own_r0 = (t0 - (0 if tab == 0 else NTILES_A)) * 128
        all_r0 = AG_BASE[c] - (0 if tab == 0 else HALFR)
        ag_meta.append((t0 + nt - 1, tab, own_r0, all_r0, nt * 128))

    with tile.TileContext(nc) as tc:
        with (
            tc.tile_pool(name="const", bufs=1) as cpool,
            tc.tile_pool(name="tabw", bufs=2) as tabw,
            tc.tile_pool(name="ld2", bufs=2) as ld2,
            tc.tile_pool(name="gath", bufs=4) as gpool,
            tc.tile_pool(name="work", bufs=2) as wpool,
            tc.tile_pool(name="fin", bufs=2) as fpool,
            tc.tile_pool(name="fint", bufs=4) as ftpool,
            tc.tile_pool(name="psu", bufs=2, space="PSUM") as psu,
            tc.tile_pool(name="psx", bufs=2, space="PSUM") as psx,
            tc.tile_pool(name="psT", bufs=2, space="PSUM") as psT,
        ):
            # ---------- persistent constants in SBUF
            def load_const(t, shape, dt):
                tl = cpool.tile(shape, dt, tag=t.name, name=t.name + "_sb")
                nc.sync.dma_start(out=tl[:], in_=t[:])
                return tl

            wl1_sb = load_const(wl1, [128, HC], BF16)
            wr1_sb = load_const(wr1, [128, HC], BF16)
            w4_sb = load_const(w4, [128, OUT_F], BF16)
            iavT1_sb = load_const(iavT1, [128, 2], F32)
            iavT2_sb = load_const(iavT2, [128, 2], F32)
            bT1_sb = load_const(bT1, [128, 2], F32)
            bT2_sb = load_const(bT2, [128, 2], F32)
            b3c_sb = load_const(b3c, [128, 1], F32)
            b4f_sb = load_const(b4f, [128, OUT_F], F32)
            iden_sb = load_const(idenBF, [128, 128], BF16)
            epsc_sb = load_const(epsc, [128, 1], F32)


            def load_const2(t, cols, tag):
                tl = cpool.tile([128, 2, cols], BF16, tag=tag, name=tag + "_sb")
                nc.sync.dma_start(
                    out=tl[:], in_=t.rearrange("(a p) c -> p a c", p=128))
                return tl

            wl2_sb = load_const2(wl2, HC, "wl2x")
            wr2_sb = load_const2(wr2, HC, "wr2x")
            w3_sb = load_const2(w3, 128, "w3x")

            def own_slice(owna, ownb, t):
                if t < NTILES_A:
                    return owna[t * 128:(t + 1) * 128, :]
                tb = t - NTILES_A
                return ownb[tb * 128:(tb + 1) * 128, :]

            def all_slice(alla, allb, gt):
                if gt < HALFR // 128:
                    return alla[gt * 128:(gt + 1) * 128, :]
                gb_ = gt - HALFR // 128
                return allb[gb_ * 128:(gb_ + 1) * 128, :]

            def ag_fire(owns, alls, t):
                """Fire any AllGather chunk whose last tile is t."""
                for (lt, tab, own_r0, all_r0, nr) in ag_meta:
                    if lt != t:
                        continue
                    nc.gpsimd.collective_compute(
                        "AllGather", mybir.AluOpType.bypass,
                        replica_groups=[list(range(NCORES))],
                        ins=[owns[tab][own_r0:own_r0 + nr, :]],
                        outs=[alls[tab][all_r0:all_r0 + nr * NCORES, :]])

            # ---------- L1 tables, replicated: every core computes the FULL
            # xl1 gather table from x (no collective), plus its own loc1.
            def table_local_l1():
                for t in range(NTILES):
                    lt = tabw.tile([128, 128], BF16, tag="tablhs")
                    nc.sync.dma_start(out=lt[:],
                                      in_=xoT[:, t * 128:(t + 1) * 128])
                    ot = tabw.tile([128, 2, HC], BF16, tag="tabout")
                    for j, w_sb in ((0, wl1_sb), (1, wr1_sb)):
                        pst = psu.tile([128, 258], F32, tag="u")
                        nc.tensor.matmul(pst[:, 0:HC], lt[:], w_sb[:],
                                         start=True, stop=True)
                        if j == 0:
                            nc.vector.tensor_copy(ot[:, j, :], pst[:, 0:HC])
                        else:
                            nc.scalar.activation(ot[:, j, :], pst[:, 0:HC],
                                                 AF.Copy)
                    nc.scalar.dma_start(
                        out=loc1[t * 128:(t + 1) * 128, :, :], in_=ot[:])

            def table_full_l1():
                TG = 8
                for g0 in range(0, GTILES, TG):
                    lt = tabw.tile([128, TG * 128], BF16, tag="tabghs")
                    nc.sync.dma_start(out=lt[:],
                                      in_=xgT[:, g0 * 128:(g0 + TG) * 128])
                    og = tabw.tile([128, TG, HC], BF16, tag="tabgo")
                    for c in range(2):
                        pst = psx.tile([128, 4, HC], F32, tag="xrb")
                        for j in range(4):
                            nc.tensor.matmul(
                                pst[:, j, :],
                                lt[:, (c * 4 + j) * 128:(c * 4 + j + 1) * 128],
                                wl1_sb[:], start=True, stop=True)
                        if c == 0:
                            nc.vector.tensor_copy(og[:, 0:4, :], pst[:])
                        else:
                            nc.scalar.activation(og[:, 4:8, :], pst[:],
                                                 AF.Copy)
                    # groups of 8 tiles never straddle the A/B split;
                    # lane-major layout: row = p*ntt + t (4 KB contiguous
                    # per partition per group)
                    if g0 < HALFR // 128:
                        dst_t, t0g, ntt = xl_all1a, g0, HALFR // 128
                    else:
                        dst_t, t0g, ntt = (xl_all1b, g0 - HALFR // 128,
                                           (S - HALFR) // 128)
                    nc.sync.dma_start(
                        out=dst_t.rearrange("(p tt) c -> p tt c", p=128)[
                            :, t0g:t0g + TG, :],
                        in_=og[:])

            # ---------- edge phase (one conv layer), software-pipelined
            def conv_layer(xl_ta, xl_tb, ixta, ixtb, loc_tab, qf,
                           fin_pre_cb, fin_post_cb, ag_cb=None):
                n_batches = NTILES // GB + (1 if NTILES % GB else 0)
                st = {}

                def nb_of(bi):
                    return min(bi * GB + GB, NTILES) - bi * GB

                def stage_gather(bi):
                    t0, nb = bi * GB, nb_of(bi)
                    s = st.setdefault(bi, {})
                    ixa = ld2.tile([128, GB, TA * 8], I16, tag="ixa")
                    nc.sync.dma_start(
                        out=ixa[:, 0:nb],
                        in_=ixta[t0:t0 + nb].rearrange("t p c -> p t c"))
                    ixb = ld2.tile([128, GB, TB * 8], I16, tag="ixb")
                    nc.sync.dma_start(
                        out=ixb[:, 0:nb],
                        in_=ixtb[t0:t0 + nb].rearrange("t p c -> p t c"))
                    s["gA"] = gpool.tile([128, GB * TA, HC], BF16, tag="gA", name="gA_sb")
                    s["gB"] = gpool.tile([128, GB * TB, HC], BF16, tag="gB", name="gB_sb")
                    nsa = nb * TA
                    ixa_f = ixa[:, 0:nb].rearrange("p t c -> p (t c)")
                    ixb_f = ixb[:, 0:nb].rearrange("p t c -> p (t c)")
                    for g, ixf, tab, q0 in ((s["gA"], ixa_f, xl_ta, 0),
                                            (s["gB"], ixb_f, xl_tb, 2)):
                        h = nsa // 2
                        nc.gpsimd.dma_gather(
                            out_ap=g[:, 0:h, :], in_ap=tab[:, :],
                            idxs_ap=ixf[:, 0:h * 8],
                            num_idxs=h * 128, num_idxs_reg=h * 128,
                            elem_size=HC, single_packet=False, queue_num=q0)
                        nc.gpsimd.dma_gather(
                            out_ap=g[:, h:nsa, :], in_ap=tab[:, :],
                            idxs_ap=ixf[:, h * 8:nsa * 8],
                            num_idxs=(nsa - h) * 128,
                            num_idxs_reg=(nsa - h) * 128,
                            elem_size=HC, single_packet=False,
                            queue_num=q0 + 1)

                def stage_load_mkT(bi):
                    t0, nb = bi * GB, nb_of(bi)
                    s = st.setdefault(bi, {})
                    s["mkT"] = ld2.tile([128, GB, TS * 128], BF16, tag="mkT", name="mkT_sb")
                    nc.sync.dma_start(
                        out=s["mkT"][:, 0:nb],
                        in_=mkTd[t0:t0 + nb].rearrange("t p c -> p t c"))

                def stage_load_near(bi):
                    t0, nb = bi * GB, nb_of(bi)
                    s = st.setdefault(bi, {})
                    s["sxb"] = ld2.tile([128, GB, 2, HC], BF16, tag="sxb", name="sxb_sb")
                    nc.sync.dma_start(
                        out=s["sxb"][:, 0:nb],
                        in_=loc_tab[t0 * 128:(t0 + nb) * 128].rearrange(
                            "(a p) b c -> p a b c", p=128))
                    s["mk"] = ld2.tile([128, GB, NS * 128], BF16, tag="mk", name="mk_sb")
                    nc.sync.dma_start(
                        out=s["mk"][:, 0:nb],
                        in_=mkd[t0:t0 + nb].rearrange("t p c -> p t c"))

                def stage_u(bi):
                    nb = nb_of(bi)
                    s = st[bi]
                    gA, gB, sxb, mkT = s["gA"], s["gB"], s["sxb"], s["mkT"]
                    work = wpool.tile([128, GB * NS, 258], BF16, tag="work")
                    s["work"] = work
                    w4d = work[:, :, 0:HC].rearrange(
                        "p (t s) c -> p t s c", s=NS)
                    s["w4d"] = w4d
                    # u = xl[src] + xr[dst]: xr via one-hot matmul, xl via
                    # identity-matmul accumulate, ACT drains chunks to bf16.
                    for ti in range(nb):
                        for s0 in range(0, TS, 4):
                            s1 = min(s0 + 4, TS)
                            xrb = psx.tile([128, 4, HC], F32, tag="xrb")
                            for si in range(s0, s1):
                                nc.tensor.matmul(
                                    xrb[:, si - s0, :],
                                    mkT[:, ti, si * 128:(si + 1) * 128],
                                    sxb[:, ti, 1, :], start=True, stop=False)
                                g, gofs = (gA, 0) if si < TA else (gB, TA)
                                nc.tensor.matmul(
                                    xrb[:, si - s0, :], iden_sb[:],
                                    g[:, ti * TA + si - gofs, :],
                                    start=False, stop=True)
                            nc.scalar.activation(
                                w4d[:, ti, s0:s1, :], xrb[:, 0:s1 - s0, :],
                                AF.Prelu, alpha=SLOPE)
                    # self subtile: loc xl + xr, then leaky (max form —
                    # tables are |att|-scaled so every column is max-form)
                    nc.vector.tensor_tensor(
                        out=w4d[:, 0:nb, TS, :],
                        in0=sxb[:, 0:nb, 0, :], in1=sxb[:, 0:nb, 1, :],
                        op=OP.add)
                    nc.vector.scalar_tensor_tensor(
                        out=w4d[:, 0:nb, TS, :], in0=w4d[:, 0:nb, TS, :],
                        scalar=SLOPE, in1=w4d[:, 0:nb, TS, :],
                        op0=OP.mult, op1=OP.max)

                def stage_score(bi):
                    nb = nb_of(bi)
                    s = st[bi]
                    qs, flips = qf
                    work = s["work"]
                    wf = work[:, 0:nb * NS]
                    wh = wf[:, :, 0:HC].rearrange("p s (h c) -> p s h c", h=2)
                    # flip the minority-sign members of each head's single
                    # mixed quad, then the fold tree is sign-pure per column
                    for h in range(2):
                        for c in flips[h]:
                            nc.vector.tensor_scalar(
                                out=wh[:, :, h, c:c + 1],
                                in0=wh[:, :, h, c:c + 1],
                                scalar1=-1.0, scalar2=None, op0=OP.mult)
                    nc.vector.tensor_tensor(
                        out=wh[:, :, :, 0:64], in0=wh[:, :, :, 0:64],
                        in1=wh[:, :, :, 64:128], op=OP.add)
                    nc.vector.tensor_tensor(
                        out=wh[:, :, :, 0:32], in0=wh[:, :, :, 0:32],
                        in1=wh[:, :, :, 32:64], op=OP.add)
                    # score = sum(pos finals) - sum(neg finals) per head
                    sc = wpool.tile([128, GB * NS, 2], F32, tag="sc")
                    sn = wpool.tile([128, GB * NS, 2], F32, tag="sn")
                    for h in range(2):
                        q = qs[h]
                        nc.vector.tensor_reduce(
                            out=sc[:, 0:nb * NS, h:h + 1].rearrange(
                                "p s h -> p s h ()"),
                            in_=wh[:, :, h:h + 1, 0:q],
                            axis=mybir.AxisListType.X, op=OP.add)
                        nc.vector.tensor_reduce(
                            out=sn[:, 0:nb * NS, h:h + 1].rearrange(
                                "p s h -> p s h ()"),
                            in_=wh[:, :, h:h + 1, q:32],
                            axis=mybir.AxisListType.X, op=OP.add)
                    nc.vector.tensor_tensor(
                        out=sc[:, 0:nb * NS], in0=sc[:, 0:nb * NS],
                        in1=sn[:, 0:nb * NS], op=OP.subtract)
                    af = wpool.tile([128, GB * NS, 2], F32, tag="af")
                    s["af"] = af
                    nc.scalar.activation(af[:, 0:nb * NS], sc[:, 0:nb * NS],
                                         AF.Exp)
                    nc.scalar.activation(work[:, 0:nb * NS, HC:HC + 2],
                                         af[:, 0:nb * NS], AF.Copy)

                def stage_y(bi):
                    nb = nb_of(bi)
                    s = st[bi]
                    gA, gB, sxb, w4d = s["gA"], s["gB"], s["sxb"], s["w4d"]
                    af = s["af"]
                    for ti in range(nb):
                        for h in range(2):
                            ab = af[:, ti * NS:(ti + 1) * NS,
                                    h:h + 1].broadcast_to([128, NS, 128])
                            cl, ch = h * 128, h * 128 + 128
                            nc.vector.tensor_tensor(
                                out=w4d[:, ti, 0:TA, cl:ch],
                                in0=gA[:, ti * TA:(ti + 1) * TA, cl:ch],
                                in1=ab[:, 0:TA], op=OP.mult)
                            nc.vector.tensor_tensor(
                                out=w4d[:, ti, TA:TS, cl:ch],
                                in0=gB[:, ti * TB:(ti + 1) * TB, cl:ch],
                                in1=ab[:, TA:TS], op=OP.mult)
                            nc.vector.tensor_tensor(
                                out=w4d[:, ti, TS, cl:ch],
                                in0=sxb[:, ti, 0, cl:ch],
                                in1=ab[:, TS], op=OP.mult)

                def stage_scatter(bi):
                    t0, nb = bi * GB, nb_of(bi)
                    s = st[bi]
                    mk, work = s["mk"], s["work"]
                    pres = []
                    for ti in range(nb):
                        u_ps = psu.tile([128, 258], F32, tag="u")
                        for si in range(NS):
                            nc.tensor.matmul(
                                u_ps[:], mk[:, ti, si * 128:(si + 1) * 128],
                                work[:, ti * NS + si, :],
                                start=(si == 0), stop=(si == NS - 1))
                        pres.append(fin_pre_cb(u_ps))
                    s["pres"] = pres

                def stage_finpost(bi):
                    t0, nb = bi * GB, nb_of(bi)
                    pres = st[bi]["pres"]
                    for ti in range(nb):
                        fin_post_cb(t0 + ti, pres[ti])
                        if ag_cb is not None:
                            ag_cb(t0 + ti)
                    del st[bi]

                # ---- pipelined schedule: gathers dispatched 3 batches
                # ahead, masks/sxb loaded 1-2 ahead, fin writes on ACT-DMA
                for bj in range(min(3, n_batches)):
                    stage_gather(bj)
                stage_load_mkT(0)
                if n_batches > 1:
                    stage_load_mkT(1)
                stage_load_near(0)
                stage_u(0)
                for bi in range(n_batches):
                    if bi + 1 < n_batches:
                        stage_load_near(bi + 1)
                    stage_score(bi)
                    if bi > 0:
                        stage_finpost(bi - 1)
                    if bi + 3 < n_batches:
                        stage_gather(bi + 3)
                    if bi + 2 < n_batches:
                        stage_load_mkT(bi + 2)
                    if bi + 1 < n_batches:
                        stage_u(bi + 1)
                    stage_y(bi)
                    stage_scatter(bi)
                stage_finpost(n_batches - 1)

            # ---------- finalize, split so PE never stalls behind the
            # ACT normalization chain: fin_pre (ACT/DVE, frees u_ps) then
            # fin_post (PE transposes + dense tail) after the next scatters.
            def fin_pre(u_ps, iavT_sb, bT_sb):
                dcol = fpool.tile([128, 2], F32, tag="dcol")
                nc.vector.tensor_scalar(
                    out=dcol[:], in0=u_ps[:, HC:HC + 2],
                    scalar1=epsc_sb[:, 0:1], scalar2=None, op0=OP.add)
                rcol = fpool.tile([128, 2], F32, tag="rcol")
                nc.vector.reciprocal(rcol[:], dcol[:])
                t1 = ftpool.tile([128, 2, 128], BF16, tag="t1")
                for h in range(2):
                    nc.vector.tensor_scalar(
                        out=t1[:, h, :], in0=u_ps[:, h * 128:(h + 1) * 128],
                        scalar1=rcol[:, h:h + 1], scalar2=None, op0=OP.mult)
                return t1

            def fin_tc(t1, iavT_sb, bT_sb):
                pt = psT.tile([128, 2, 128], BF16, tag="fps")
                for h in range(2):
                    nc.tensor.transpose(pt[:, h, :], t1[:, h, :], iden_sb[:])
                cts = []
                for h in range(2):
                    ct = fpool.tile([128, 128], BF16, tag=f"ct{h}")
                    nc.scalar.activation(ct[:], pt[:, h, :], AF.Relu,
                                         scale=iavT_sb[:, h:h + 1],
                                         bias=bT_sb[:, h:h + 1])
                    cts.append(ct)
                return cts

            def fin1_pre(u_ps):
                return fin_pre(u_ps, iavT1_sb, bT1_sb)

            def fin1_post(t, t1):
                cts = fin_tc(t1, iavT1_sb, bT1_sb)
                ot2 = fpool.tile([128, 2, HC], BF16, tag="ot2")
                for j, w2_sb in ((0, wl2_sb), (1, wr2_sb)):
                    pst = psu.tile([128, 258], F32, tag="u")
                    nc.tensor.matmul(pst[:, 0:HC], cts[0][:], w2_sb[:, 0, :],
                                     start=True, stop=False)
                    nc.tensor.matmul(pst[:, 0:HC], cts[1][:], w2_sb[:, 1, :],
                                     start=False, stop=True)
                    nc.scalar.activation(ot2[:, j, :], pst[:, 0:HC], AF.Copy)
                nc.scalar.dma_start(
                    out=loc2[t * 128:(t + 1) * 128, :, :], in_=ot2[:])
                nc.scalar.dma_start(
                    out=own_slice(xl_own2a, xl_own2b, t), in_=ot2[:, 0, :])

            def fin2_pre(u_ps):
                return fin_pre(u_ps, iavT2_sb, bT2_sb)

            def fin2_post(t, t1):
                cts = fin_tc(t1, iavT2_sb, bT2_sb)
                zt_ps = psu.tile([128, 258], F32, tag="u")
                nc.tensor.matmul(zt_ps[:, 0:128], w3_sb[:, 0, :], cts[0][:],
                                 start=True, stop=False)
                nc.tensor.matmul(zt_ps[:, 0:128], w3_sb[:, 1, :], cts[1][:],
                                 start=False, stop=True)
                zt_sb = fpool.tile([128, 128], BF16, tag="ztsb")
                nc.scalar.activation(zt_sb[:], zt_ps[:, 0:128], AF.Identity,
                                     bias=b3c_sb[:, 0:1])
                o_ps = psu.tile([128, 258], F32, tag="u")
                nc.tensor.matmul(o_ps[:, 0:OUT_F], zt_sb[:], w4_sb[:],
                                 start=True, stop=True)
                o_pre = fpool.tile([128, OUT_F], F32, tag="opre")
                nc.vector.scalar_tensor_tensor(
                    out=o_pre[:], in0=o_ps[:, 0:OUT_F], scalar=1.0,
                    in1=b4f_sb[:], op0=OP.mult, op1=OP.add)
                o_sb = fpool.tile([128, OUT_F], F32, tag="osb")
                nc.scalar.activation(o_sb[:], o_pre[:], AF.Sigmoid)
                nc.scalar.dma_start(out=out_ext[t * 128:(t + 1) * 128, :],
                                    in_=o_sb[:])

            # ================= phase schedule =================
            import os as _os
            _upto = int(_os.environ.get("KPHASES", "9"))

            table_local_l1()
            table_full_l1()
            if _upto >= 2:
                ag2 = lambda t: ag_fire((xl_own2a, xl_own2b),
                                        (xl_all2a, xl_all2b), t)
                conv_layer(xl_all1a, xl_all1b, idxXLA1, idxXLB1, loc1,
                           LR1, fin1_pre, fin1_post, ag_cb=ag2)
            if _upto >= 4:
                conv_layer(xl_all2a, xl_all2b, idxXLA, idxXLB, loc2,
                           LR2, fin2_pre, fin2_post)
            else:
                zt = fpool.tile([128, OUT_F], F32, tag="osb")
                nc.vector.memset(zt[:], 0.0)
                for t in range(NTILES):
                    nc.sync.dma_start(out=out_ext[t * 128:(t + 1) * 128, :],
                                      in_=zt[:])

    nc.compile()
    return nc


# ---------------------------------------------------------------- entry point
def kernel(**inputs):
    from concourse import bass_utils

    src = np.asarray(inputs["edge_index"][0], np.int64)
    dst = np.asarray(inputs["edge_index"][1], np.int64)
    x = np.asarray(inputs["x"], np.float32)

    pack = _pack_graph(src, dst)
    nos = pack["node_of_slot"]
    valid = nos >= 0
    x_slot = np.zeros((S, IN_F), np.float32)
    x_slot[valid] = x[nos[valid]]
    sog = pack["slot_of_grow"]
    x_grow = np.zeros((S, IN_F), np.float32)
    gv = sog >= 0
    x_grow[gv] = x_slot[sog[gv]]

    def bf(a):
        return np.ascontiguousarray(np.asarray(a, np.float32)).astype(BF)

    # --- per-head column permutation: sign-pure quads so the fold tree
    # (c, c+32, c+64, c+96 -> final col c) only combines same-sign columns;
    # score = reduce(pos finals) - reduce(neg finals). At most one mixed
    # quad per head; its minority-sign members get flipped on-device.
    def prep_layer(att):
        att = np.asarray(att, np.float32).reshape(2, 128)
        perm = np.zeros(HC, np.int64)
        qs, flips = [], []
        for h in range(2):
            a = att[h]
            pos = list(np.where(a > 0)[0])
            neg = list(np.where(a <= 0)[0])
            p = len(pos)
            np_q = p // 4
            mixed = 1 if p % 4 else 0
            colassign = np.empty((4, 32), np.int64)
            pi = ni = 0
            fl = []
            for j in range(32):
                if j < np_q:
                    for m in range(4):
                        colassign[m, j] = pos[pi]; pi += 1
                elif mixed and j == np_q:
                    for m in range(4):
                        if pi < p:
                            colassign[m, j] = pos[pi]; pi += 1
                        else:
                            colassign[m, j] = neg[ni]; ni += 1
                            fl.append(m * 32 + j)
                else:
                    for m in range(4):
                        colassign[m, j] = neg[ni]; ni += 1
            for m in range(4):
                for j in range(32):
                    perm[h * 128 + m * 32 + j] = h * 128 + colassign[m, j]
            q = np_q + mixed
            assert 0 < q < 32, f"degenerate sign split p={p}"
            qs.append(q)
            flips.append(fl)
        att_p = att.reshape(HC)[perm]
        att_p = np.where(np.abs(att_p) < 1e-30, 1e-30, att_p)
        att_p = np.abs(att_p)
        return perm, att_p, (qs, flips)

    perm1, att1p, LR1 = prep_layer(inputs["att1"])
    perm2, att2p, LR2 = prep_layer(inputs["att2"])
    _LR_RANGES["l1"] = LR1
    _LR_RANGES["l2"] = LR2

    Wl1p = np.asarray(inputs["Wl1"], np.float32)[:, perm1] * att1p[None, :]
    Wr1p = np.asarray(inputs["Wr1"], np.float32)[:, perm1] * att1p[None, :]
    Wl2p = (np.asarray(inputs["Wl2"], np.float32)[perm1][:, perm2]
            * att2p[None, :])
    Wr2p = (np.asarray(inputs["Wr2"], np.float32)[perm1][:, perm2]
            * att2p[None, :])
    W3p = np.asarray(inputs["W3"], np.float32)[perm2]
    b1p = np.asarray(inputs["b1"], np.float32)[perm1]
    b2p = np.asarray(inputs["b2"], np.float32)[perm2]

    common = {
        "wl1": bf(Wl1p), "wr1": bf(Wr1p),
        "wl2": bf(Wl2p), "wr2": bf(Wr2p),
        "w3": bf(W3p), "w4": bf(inputs["W4"]),
        "iavT1": np.ascontiguousarray(
            (1.0 / att1p).reshape(2, 128).T.astype(np.float32)),
        "iavT2": np.ascontiguousarray(
            (1.0 / att2p).reshape(2, 128).T.astype(np.float32)),
        "bT1": np.ascontiguousarray(b1p.reshape(2, 128).T.astype(np.float32)),
        "bT2": np.ascontiguousarray(b2p.reshape(2, 128).T.astype(np.float32)),
        "b3c": np.asarray(inputs["b3"], np.float32).reshape(128, 1),
        "b4f": np.tile(np.asarray(inputs["b4"], np.float32)[None, :], (128, 1)),
        "idenBF": np.eye(128, dtype=np.float32).astype(BF),
        "epsc": np.full((128, 1), 1e-16, np.float32),

        "xgT": np.ascontiguousarray(x_grow.T).astype(BF),
    }

    in_maps = []
    for k in range(NCORES):
        m = dict(common)
        m["xoT"] = np.ascontiguousarray(
            x_slot[k * SPC:(k + 1) * SPC].T).astype(BF)
        ixla = np.empty((NTILES, 128, TA * 8), np.int16)
        ixlb = np.empty((NTILES, 128, TB * 8), np.int16)
        ixla1 = np.empty((NTILES, 128, TA * 8), np.int16)
        ixlb1 = np.empty((NTILES, 128, TB * 8), np.int16)
        for t in range(NTILES):
            ixla[t] = _wrap_idx(pack["idxXL"][k, t, :TA * 128])
            ixlb[t] = _wrap_idx(pack["idxXL"][k, t, TA * 128:])
            ixla1[t] = _wrap_idx(pack["idxXL1"][k, t, :TA * 128])
            ixlb1[t] = _wrap_idx(pack["idxXL1"][k, t, TA * 128:])
        m["idxXLA"] = ixla
        m["idxXLB"] = ixlb
        m["idxXLA1"] = ixla1
        m["idxXLB1"] = ixlb1
        m["mkd"] = pack["mk"][k].astype(BF)
        m["mkTd"] = pack["mkT"][k].astype(BF)
        in_maps.append(m)

    if "nc" not in _NC_CACHE:
        _NC_CACHE["nc"] = _build_nc()
    nc = _NC_CACHE["nc"]

    res = bass_utils.run_bass_kernel_spmd(nc, in_maps,
                                          core_ids=list(range(NCORES)),
                                          **_RUN_OPTS)
    _LAST_RESULTS["res"] = res
    out_slots = np.concatenate([res.results[k]["out"] for k in range(NCORES)], 0)
    return out_slots[pack["slot_of_node"]].astype(np.float32)


# revision 38
# speedup vs baseline: 1.1589x; 1.0520x over previous
"""GATv2 (2-layer, 2-head) Trainium2 kernel, 8-core SPMD — v5.

vs v4: layer-1 xl table computed redundantly on every core (no L1
AllGather, fast startup), conv batches software-pipelined (u-phase of
batch b+1 issues before score/y/scatter of batch b so PE never waits
behind the DVE/ACT chain), DMA loads prefetch 2 batches ahead, leaky
back to single scalar_tensor_tensor, y fully on DVE.
"""
import sys

sys.path.insert(0, "/opt/trn_rl_repo")

import numpy as np
import ml_dtypes

BF = ml_dtypes.bfloat16

# ---- static layout constants (match reference problem sizes) ----
N = 50000
NCORES = 8
LANES = 128
NTILES = 49
SPC = NTILES * LANES          # 6272 slots per core
S = NCORES * SPC              # 50176 total slots
GTILES = S // 128             # 392 gather-table tiles
TA = 7                        # table-A gather subtiles per dst-tile
TB = 7
TS = TA + TB                  # random-edge subtiles (self subtile is extra)
NS = TS + 1                   # subtiles per tile incl self
GB = 3                        # dst-tiles per gather batch
IN_F = 128
HC = 256                      # H*C
OUT_F = 40
SLOPE = 0.2
# AllGather chunking (layer 2 only): 5 tile groups (sum = NTILES).
# Groups 0,1 make up gather table A, groups 2,3,4 table B.
AG_CH = (16, 8, 12, 8, 5)
AG_T0 = (0, 16, 24, 36, 44)
AG_TAB = (0, 0, 1, 1, 1)
NTILES_A = 24                 # tiles in table A
HALFR = NTILES_A * LANES * NCORES   # 24576 rows in table A
_b = [0] * len(AG_CH)
_acc = [0, 0]
for _c in range(len(AG_CH)):
    _b[_c] = (0 if AG_TAB[_c] == 0 else HALFR) + _acc[AG_TAB[_c]]
    _acc[AG_TAB[_c]] += AG_CH[_c] * LANES * NCORES
AG_BASE = tuple(_b)

_NC_CACHE = {}
_RUN_OPTS = {}
_LAST_RESULTS = {}
_LR_RANGES = {}


# ---------------------------------------------------------------- host prep
def _pack_graph(src, dst):
    deg = np.bincount(dst, minlength=N)

    is_self = src == dst
    self_eids = np.full(N, -1, np.int64)
    sids = np.where(is_self)[0]
    self_eids[src[sids]] = sids
    rand_mask = np.ones(len(src), bool)
    rand_mask[self_eids[self_eids >= 0]] = False

    nodes_per_core = (N + NCORES - 1) // NCORES
    order = np.argsort(-deg, kind="stable")
    core_edges = np.zeros(NCORES, np.int64)
    core_nodes = np.zeros(NCORES, np.int64)
    core_of_node = np.full(N, -1, np.int32)
    for v in order:
        k = np.argmin(np.where(core_nodes < nodes_per_core, core_edges, 1 << 60))
        core_of_node[v] = k
        core_edges[k] += deg[v]
        core_nodes[k] += 1

    rsrc, rdst = src[rand_mask], dst[rand_mask]

    # --- chunk-group assignment per core (before tile packing): deal nodes
    # round-robin by out-degree so the gather-table halves stay balanced.
    NG = len(AG_CH)
    odeg = np.bincount(rsrc, minlength=N)
    group_of_node = np.full(N, -1, np.int8)
    gcap = [c * LANES for c in AG_CH]
    for k in range(NCORES):
        vs = np.where(core_of_node == k)[0]
        vs = vs[np.argsort(-odeg[vs], kind="stable")]
        cnt = [0] * NG
        gi = 0
        for v in vs:
            while cnt[gi % NG] >= gcap[gi % NG]:
                gi += 1
            group_of_node[v] = gi % NG
            cnt[gi % NG] += 1
            gi += 1
    eh_node = np.asarray(AG_TAB, np.int8)[group_of_node]

    dA = np.bincount(rdst[eh_node[rsrc] == 0], minlength=N)
    dB = np.bincount(rdst[eh_node[rsrc] == 1], minlength=N)
    capA, capB = TA * LANES, TB * LANES

    tile_of_node = np.full(N, -1, np.int32)
    lane_of_node = np.full(N, -1, np.int32)
    for k in range(NCORES):
        for g in range(NG):
            vs = np.where((core_of_node == k) & (group_of_node == g))[0]
            vs = vs[np.argsort(-(dA[vs] + dB[vs]), kind="stable")]
            nv = len(vs)
            ntg = AG_CH[g]
            tile = np.empty(nv, np.int64)
            for i in range(nv):
                r, c = divmod(i, ntg)
                tile[i] = c if r % 2 == 0 else ntg - 1 - c
            loadA = np.bincount(tile, weights=dA[vs],
                                minlength=ntg).astype(np.int64)
            loadB = np.bincount(tile, weights=dB[vs],
                                minlength=ntg).astype(np.int64)
            it = 0
            while (loadA.max() > capA or loadB.max() > capB) and it < 100000:
                it += 1
                t_bad = int(np.argmax(np.maximum(loadA - capA, loadB - capB)))
                overA = loadA[t_bad] - capA >= loadB[t_bad] - capB
                t_good = int(np.argmin(loadA + loadB))
                in_bad = np.where(tile == t_bad)[0]
                in_good = np.where(tile == t_good)[0]
                d_bad = dA[vs[in_bad]] if overA else dB[vs[in_bad]]
                ib = in_bad[np.argmax(d_bad)]
                ig = in_good[np.argmin(dA[vs[in_good]] + dB[vs[in_good]])]
                for i, frm, to in ((ib, t_bad, t_good), (ig, t_good, t_bad)):
                    v = vs[i]
                    tile[i] = to
                    loadA[frm] -= dA[v]; loadA[to] += dA[v]
                    loadB[frm] -= dB[v]; loadB[to] += dB[v]
            if loadA.max() > capA or loadB.max() > capB:
                raise RuntimeError("edge packing failed; need bigger TA/TB")
            tile_of_node[vs] = AG_T0[g] + tile
            for t in range(ntg):
                nodes_t = vs[tile == t]
                lane_of_node[nodes_t] = np.arange(len(nodes_t))

    slot_of_node = (core_of_node.astype(np.int64) * SPC
                    + tile_of_node * LANES + lane_of_node)
    node_of_slot = np.full(S, -1, np.int64)
    node_of_slot[slot_of_node] = np.arange(N)

    # chunk-major gather-table row of each node
    g_arr = group_of_node.astype(np.int64)
    base = np.asarray(AG_BASE, np.int64)[g_arr]
    t0 = np.asarray(AG_T0, np.int64)[g_arr]
    chw = np.asarray(AG_CH, np.int64)[g_arr]
    grow_of_node = (base + core_of_node * chw * LANES
                    + (tile_of_node - t0) * LANES + lane_of_node)

    srcrow = grow_of_node[rsrc]
    dstslot = slot_of_node[rdst]
    dst_core = (dstslot // SPC).astype(np.int32)
    dst_tile = ((dstslot % SPC) // LANES).astype(np.int32)
    dst_lane = (dstslot % LANES).astype(np.int32)
    eh = (srcrow >= HALFR).astype(np.int8)

    idxXL = np.zeros((NCORES, NTILES, TS * 128), np.int16)
    idxXL1 = np.zeros((NCORES, NTILES, TS * 128), np.int16)

    key = (dst_core.astype(np.int64) * NTILES + dst_tile) * 2 + eh
    es = np.argsort(key, kind="stable")
    ksrc = srcrow[es]; kdl = dst_lane[es]
    kc = dst_core[es]; kt = dst_tile[es]; kh = eh[es]
    gkey = key[es]
    start = np.zeros(len(es), bool)
    start[0] = True
    start[1:] = gkey[1:] != gkey[:-1]
    gs = np.where(start, np.arange(len(es)), 0)
    gidx = np.arange(len(es)) - np.maximum.accumulate(gs)
    off = np.where(kh == 0, 0, TA * 128) + gidx
    tabrow = np.where(kh == 0, ksrc, ksrc - HALFR).astype(np.int64)
    idxXL[kc, kt, off] = tabrow.astype(np.int16)
    # conv1 gathers read the lane-major replicated L1 table:
    # row' = lane*(tiles in table) + tile
    ntt = np.where(kh == 0, HALFR // 128, (S - HALFR) // 128)
    row1 = (tabrow % 128) * ntt + tabrow // 128
    idxXL1[kc, kt, off] = row1.astype(np.int16)

    # one-hot masks: mk [e-lane -> dst-lane] per subtile (incl self at TS),
    # mkT [dst-lane -> e-lane] per random subtile.
    ksi = (off // 128).astype(np.int64)
    kel = (off % 128).astype(np.int64)
    mk = np.zeros((NCORES, NTILES, 128, NS * 128), np.float32)
    mkT = np.zeros((NCORES, NTILES, 128, TS * 128), np.float32)
    mk[kc, kt, kel, ksi * 128 + kdl] = 1.0
    mkT[kc, kt, kdl, ksi * 128 + kel] = 1.0
    vsel = np.where(self_eids >= 0)[0]
    ln = lane_of_node[vsel].astype(np.int64)
    mk[core_of_node[vsel], tile_of_node[vsel], ln, TS * 128 + ln] = 1.0

    # grow-order slot map (for the replicated L1 table build)
    slot_of_grow = np.full(S, -1, np.int64)
    slot_of_grow[grow_of_node[np.arange(N)]] = slot_of_node

    return dict(slot_of_node=slot_of_node, node_of_slot=node_of_slot,
                idxXL=idxXL, idxXL1=idxXL1, mk=mk, mkT=mkT,
                slot_of_grow=slot_of_grow)


def _wrap_idx(idx):
    """[n] -> [128, n//16] wrapped (j at partition j%16, col j//16) + replicated."""
    n = idx.shape[0]
    a = idx.reshape(n // 16, 16).T.astype(np.int16)
    return np.tile(a, (8, 1))


# ---------------------------------------------------------------- device kernel
def _build_nc():
    import concourse.bass as bass
    import concourse.bacc as bacc
    import concourse.tile as tile
    import concourse.mybir as mybir

    F32 = mybir.dt.float32
    BF16 = mybir.dt.bfloat16
    I16 = mybir.dt.int16
    AF = mybir.ActivationFunctionType
    OP = mybir.AluOpType

    LR1, LR2 = _LR_RANGES["l1"], _LR_RANGES["l2"]
    nc = bacc.Bacc(None, target_bir_lowering=False, num_swdge_queues=4)

    # ---- inputs
    xoT = nc.dram_tensor("xoT", [128, SPC], BF16, kind="ExternalInput")
    xgT = nc.dram_tensor("xgT", [128, S], BF16, kind="ExternalInput")
    wl1 = nc.dram_tensor("wl1", [128, HC], BF16, kind="ExternalInput")
    wr1 = nc.dram_tensor("wr1", [128, HC], BF16, kind="ExternalInput")
    wl2 = nc.dram_tensor("wl2", [HC, HC], BF16, kind="ExternalInput")
    wr2 = nc.dram_tensor("wr2", [HC, HC], BF16, kind="ExternalInput")
    w3 = nc.dram_tensor("w3", [HC, 128], BF16, kind="ExternalInput")
    w4 = nc.dram_tensor("w4", [128, OUT_F], BF16, kind="ExternalInput")
    iavT1 = nc.dram_tensor("iavT1", [128, 2], F32, kind="ExternalInput")
    iavT2 = nc.dram_tensor("iavT2", [128, 2], F32, kind="ExternalInput")
    bT1 = nc.dram_tensor("bT1", [128, 2], F32, kind="ExternalInput")
    bT2 = nc.dram_tensor("bT2", [128, 2], F32, kind="ExternalInput")
    b3c = nc.dram_tensor("b3c", [128, 1], F32, kind="ExternalInput")
    b4f = nc.dram_tensor("b4f", [128, OUT_F], F32, kind="ExternalInput")
    idenBF = nc.dram_tensor("idenBF", [128, 128], BF16, kind="ExternalInput")
    epsc = nc.dram_tensor("epsc", [128, 1], F32, kind="ExternalInput")

    idxXLA = nc.dram_tensor("idxXLA", [NTILES, 128, TA * 8], I16,
                            kind="ExternalInput")
    idxXLB = nc.dram_tensor("idxXLB", [NTILES, 128, TB * 8], I16,
                            kind="ExternalInput")
    idxXLA1 = nc.dram_tensor("idxXLA1", [NTILES, 128, TA * 8], I16,
                             kind="ExternalInput")
    idxXLB1 = nc.dram_tensor("idxXLB1", [NTILES, 128, TB * 8], I16,
                             kind="ExternalInput")
    mkd = nc.dram_tensor("mkd", [NTILES, 128, NS * 128], BF16,
                         kind="ExternalInput")
    mkTd = nc.dram_tensor("mkTd", [NTILES, 128, TS * 128], BF16,
                          kind="ExternalInput")
    out_ext = nc.dram_tensor("out", [SPC, OUT_F], F32, kind="ExternalOutput")

    # ---- DRAM intermediates (a/b = gather table split at tile 24)
    RA = NTILES_A * 128           # own rows in table a (3072)
    RB = (NTILES - NTILES_A) * 128  # own rows in table b (3200)
    loc1 = nc.dram_tensor("loc1", [SPC, 2, HC], BF16)
    loc2 = nc.dram_tensor("loc2", [SPC, 2, HC], BF16)
    xl_all1a = nc.dram_tensor("xl_all1a", [HALFR, HC], BF16)
    xl_all1b = nc.dram_tensor("xl_all1b", [S - HALFR, HC], BF16)
    xl_own2a = nc.dram_tensor("xl_own2a", [RA, HC], BF16)
    xl_own2b = nc.dram_tensor("xl_own2b", [RB, HC], BF16)
    xl_all2a = nc.dram_tensor("xl_all2a", [HALFR, HC], BF16,
                              addr_space="Shared")
    xl_all2b = nc.dram_tensor("xl_all2b", [S - HALFR, HC], BF16,
                              addr_space="Shared")

    # per-chunk AllGather metadata
    ag_meta = []
    for c in range(len(AG_CH)):
        t0, nt, tab = AG_T0[c], AG_CH[c], AG_TAB[c]
        own_r0 = (t0 - (0 if tab == 0 else NTILES_A)) * 128
        all_r0 = AG_BASE[c] - (0 if tab == 0 else HALFR)
        ag_meta.append((t0 + nt - 1, tab, own_r0, all_r0, nt * 128))

    with tile.TileContext(nc) as tc:
        with (
            tc.tile_pool(name="const", bufs=1) as cpool,
            tc.tile_pool(name="tabw", bufs=2) as tabw,
            tc.tile_pool(name="ld2", bufs=2) as ld2,
            tc.tile_pool(name="gath", bufs=4) as gpool,
            tc.tile_pool(name="work", bufs=2) as wpool,
            tc.tile_pool(name="fin", bufs=2) as fpool,
            tc.tile_pool(name="fint", bufs=4) as ftpool,
            tc.tile_pool(name="psu", bufs=2, space="PSUM") as psu,
            tc.tile_pool(name="psx", bufs=2, space="PSUM") as psx,
            tc.tile_pool(name="psT", bufs=2, space="PSUM") as psT,
        ):
            # ---------- persistent constants in SBUF
            def load_const(t, shape, dt):
                tl = cpool.tile(shape, dt, tag=t.name, name=t.name + "_sb")
                nc.sync.dma_start(out=tl[:], in_=t[:])
                return tl

            wl1_sb = load_const(wl1, [128, HC], BF16)
            wr1_sb = load_const(wr1, [128, HC], BF16)
            w4_sb = load_const(w4, [128, OUT_F], BF16)
            iavT1_sb = load_const(iavT1, [128, 2], F32)
            iavT2_sb = load_const(iavT2, [128, 2], F32)
            bT1_sb = load_const(bT1, [128, 2], F32)
            bT2_sb = load_const(bT2, [128, 2], F32)
            b3c_sb = load_const(b3c, [128, 1], F32)
            b4f_sb = load_const(b4f, [128, OUT_F], F32)
            iden_sb = load_const(idenBF, [128, 128], BF16)
            epsc_sb = load_const(epsc, [128, 1], F32)


            def load_const2(t, cols, tag):
                tl = cpool.tile([128, 2, cols], BF16, tag=tag, name=tag + "_sb")
                nc.sync.dma_start(
                    out=tl[:], in_=t.rearrange("(a p) c -> p a c", p=128))
                return tl

            wl2_sb = load_const2(wl2, HC, "wl2x")
            wr2_sb = load_const2(wr2, HC, "wr2x")
            w3_sb = load_const2(w3, 128, "w3x")

            def own_slice(owna, ownb, t):
                if t < NTILES_A:
                    return owna[t * 128:(t + 1) * 128, :]
                tb = t - NTILES_A
                return ownb[tb * 128:(tb + 1) * 128, :]

            def all_slice(alla, allb, gt):
                if gt < HALFR // 128:
                    return alla[gt * 128:(gt + 1) * 128, :]
                gb_ = gt - HALFR // 128
                return allb[gb_ * 128:(gb_ + 1) * 128, :]

            def ag_fire(owns, alls, t):
                """Fire any AllGather chunk whose last tile is t."""
                for (lt, tab, own_r0, all_r0, nr) in ag_meta:
                    if lt != t:
                        continue
                    nc.gpsimd.collective_compute(
                        "AllGather", mybir.AluOpType.bypass,
                        replica_groups=[list(range(NCORES))],
                        ins=[owns[tab][own_r0:own_r0 + nr, :]],
                        outs=[alls[tab][all_r0:all_r0 + nr * NCORES, :]])

            # ---------- L1 tables, replicated: every core computes the FULL
            # xl1 gather table from x (no collective), plus its own loc1.
            def table_local_l1():
                for t in range(NTILES):
                    lt = tabw.tile([128, 128], BF16, tag="tablhs")
                    nc.sync.dma_start(out=lt[:],
                                      in_=xoT[:, t * 128:(t + 1) * 128])
                    ot = tabw.tile([128, 2, HC], BF16, tag="tabout")
                    for j, w_sb in ((0, wl1_sb), (1, wr1_sb)):
                        pst = psu.tile([128, 258], F32, tag="u")
                        nc.tensor.matmul(pst[:, 0:HC], lt[:], w_sb[:],
                                         start=True, stop=True)
                        if j == 0:
                            nc.vector.tensor_copy(ot[:, j, :], pst[:, 0:HC])
                        else:
                            nc.scalar.activation(ot[:, j, :], pst[:, 0:HC],
                                                 AF.Copy)
                    nc.scalar.dma_start(
                        out=loc1[t * 128:(t + 1) * 128, :, :], in_=ot[:])

            def table_full_l1():
                TG = 8
                for g0 in range(0, GTILES, TG):
                    lt = tabw.tile([128, TG * 128], BF16, tag="tabghs")
                    nc.sync.dma_start(out=lt[:],
                                      in_=xgT[:, g0 * 128:(g0 + TG) * 128])
                    og = tabw.tile([128, TG, HC], BF16, tag="tabgo")
                    for c in range(2):
                        pst = psx.tile([128, 4, HC], F32, tag="xrb")
                        for j in range(4):
                            nc.tensor.matmul(
                                pst[:, j, :],
                                lt[:, (c * 4 + j) * 128:(c * 4 + j + 1) * 128],
                                wl1_sb[:], start=True, stop=True)
                        if c == 0:
                            nc.vector.tensor_copy(og[:, 0:4, :], pst[:])
                        else:
                            nc.scalar.activation(og[:, 4:8, :], pst[:],
                                                 AF.Copy)
                    # groups of 8 tiles never straddle the A/B split;
                    # lane-major layout: row = p*ntt + t (4 KB contiguous
                    # per partition per group)
                    if g0 < HALFR // 128:
                        dst_t, t0g, ntt = xl_all1a, g0, HALFR // 128
                    else:
                        dst_t, t0g, ntt = (xl_all1b, g0 - HALFR // 128,
                                           (S - HALFR) // 128)
                    nc.sync.dma_start(
                        out=dst_t.rearrange("(p tt) c -> p tt c", p=128)[
                            :, t0g:t0g + TG, :],
                        in_=og[:])

            # ---------- edge phase (one conv layer), software-pipelined
            def conv_layer(xl_ta, xl_tb, ixta, ixtb, loc_tab, qf,
                           fin_pre_cb, fin_post_cb, ag_cb=None):
                n_batches = NTILES // GB + (1 if NTILES % GB else 0)
                st = {}

                def nb_of(bi):
                    return min(bi * GB + GB, NTILES) - bi * GB

                def stage_gather(bi):
                    t0, nb = bi * GB, nb_of(bi)
                    s = st.setdefault(bi, {})
                    ixa = ld2.tile([128, GB, TA * 8], I16, tag="ixa")
                    nc.sync.dma_start(
                        out=ixa[:, 0:nb],
                        in_=ixta[t0:t0 + nb].rearrange("t p c -> p t c"))
                    ixb = ld2.tile([128, GB, TB * 8], I16, tag="ixb")
                    nc.sync.dma_start(
                        out=ixb[:, 0:nb],
                        in_=ixtb[t0:t0 + nb].rearrange("t p c -> p t c"))
                    s["gA"] = gpool.tile([128, GB * TA, HC], BF16, tag="gA", name="gA_sb")
                    s["gB"] = gpool.tile([128, GB * TB, HC], BF16, tag="gB", name="gB_sb")
                    nsa = nb * TA
                    ixa_f = ixa[:, 0:nb].rearrange("p t c -> p (t c)")
                    ixb_f = ixb[:, 0:nb].rearrange("p t c -> p (t c)")
                    for g, ixf, tab, q0 in ((s["gA"], ixa_f, xl_ta, 0),
                                            (s["gB"], ixb_f, xl_tb, 2)):
                        h = nsa // 2
                        nc.gpsimd.dma_gather(
                            out_ap=g[:, 0:h, :], in_ap=tab[:, :],
                            idxs_ap=ixf[:, 0:h * 8],
                            num_idxs=h * 128, num_idxs_reg=h * 128,
                            elem_size=HC, single_packet=False, queue_num=q0)
                        nc.gpsimd.dma_gather(
                            out_ap=g[:, h:nsa, :], in_ap=tab[:, :],
                            idxs_ap=ixf[:, h * 8:nsa * 8],
                            num_idxs=(nsa - h) * 128,
                            num_idxs_reg=(nsa - h) * 128,
                            elem_size=HC, single_packet=False,
                            queue_num=q0 + 1)

                def stage_load_mkT(bi):
                    t0, nb = bi * GB, nb_of(bi)
                    s = st.setdefault(bi, {})
                    s["mkT"] = ld2.tile([128, GB, TS * 128], BF16, tag="mkT", name="mkT_sb")
                    nc.sync.dma_start(
                        out=s["mkT"][:, 0:nb],
                        in_=mkTd[t0:t0 + nb].rearrange("t p c -> p t c"))

                def stage_load_near(bi):
                    t0, nb = bi * GB, nb_of(bi)
                    s = st.setdefault(bi, {})
                    s["sxb"] = ld2.tile([128, GB, 2, HC], BF16, tag="sxb", name="sxb_sb")
                    nc.sync.dma_start(
                        out=s["sxb"][:, 0:nb],
                        in_=loc_tab[t0 * 128:(t0 + nb) * 128].rearrange(
                            "(a p) b c -> p a b c", p=128))
                    s["mk"] = ld2.tile([128, GB, NS * 128], BF16, tag="mk", name="mk_sb")
                    nc.sync.dma_start(
                        out=s["mk"][:, 0:nb],
                        in_=mkd[t0:t0 + nb].rearrange("t p c -> p t c"))

                def stage_u(bi):
                    nb = nb_of(bi)
                    s = st[bi]
                    gA, gB, sxb, mkT = s["gA"], s["gB"], s["sxb"], s["mkT"]
                    work = wpool.tile([128, GB * NS, 258], BF16, tag="work")
                    s["work"] = work
                    w4d = work[:, :, 0:HC].rearrange(
                        "p (t s) c -> p t s c", s=NS)
                    s["w4d"] = w4d
                    # u = xl[src] + xr[dst]: xr via one-hot matmul, xl via
                    # identity-matmul accumulate, ACT drains chunks to bf16.
                    for ti in range(nb):
                        for s0 in range(0, TS, 4):
                            s1 = min(s0 + 4, TS)
                            xrb = psx.tile([128, 4, HC], F32, tag="xrb")
                            for si in range(s0, s1):
                                nc.tensor.matmul(
                                    xrb[:, si - s0, :],
                                    mkT[:, ti, si * 128:(si + 1) * 128],
                                    sxb[:, ti, 1, :], start=True, stop=False)
                                g, gofs = (gA, 0) if si < TA else (gB, TA)
                                nc.tensor.matmul(
                                    xrb[:, si - s0, :], iden_sb[:],
                                    g[:, ti * TA + si - gofs, :],
                                    start=False, stop=True)
                            nc.scalar.activation(
                                w4d[:, ti, s0:s1, :], xrb[:, 0:s1 - s0, :],
                                AF.Prelu, alpha=SLOPE)
                    # self subtile: loc xl + xr, then leaky (max form —
                    # tables are |att|-scaled so every column is max-form)
                    nc.vector.tensor_tensor(
                        out=w4d[:, 0:nb, TS, :],
                        in0=sxb[:, 0:nb, 0, :], in1=sxb[:, 0:nb, 1, :],
                        op=OP.add)
                    nc.vector.scalar_tensor_tensor(
                        out=w4d[:, 0:nb, TS, :], in0=w4d[:, 0:nb, TS, :],
                        scalar=SLOPE, in1=w4d[:, 0:nb, TS, :],
                        op0=OP.mult, op1=OP.max)

                def stage_score(bi):
                    nb = nb_of(bi)
                    s = st[bi]
                    qs, flips = qf
                    work = s["work"]
                    wf = work[:, 0:nb * NS]
                    wh = wf[:, :, 0:HC].rearrange("p s (h c) -> p s h c", h=2)
                    # flip the minority-sign members of each head's single
                    # mixed quad, then the fold tree is sign-pure per column
                    for h in range(2):
                        for c in flips[h]:
                            nc.vector.tensor_scalar(
                                out=wh[:, :, h, c:c + 1],
                                in0=wh[:, :, h, c:c + 1],
                                scalar1=-1.0, scalar2=None, op0=OP.mult)
                    nc.vector.tensor_tensor(
                        out=wh[:, :, :, 0:64], in0=wh[:, :, :, 0:64],
                        in1=wh[:, :, :, 64:128], op=OP.add)
                    nc.vector.tensor_tensor(
                        out=wh[:, :, :, 0:32], in0=wh[:, :, :, 0:32],
                        in1=wh[:, :, :, 32:64], op=OP.add)
                    # score = sum(pos finals) - sum(neg finals) per head
                    sc = wpool.tile([128, GB * NS, 2], F32, tag="sc")
                    sn = wpool.tile([128, GB * NS, 2], F32, tag="sn")
                    for h in range(2):
                        q = qs[h]
                        nc.vector.tensor_reduce(
                            out=sc[:, 0:nb * NS, h:h + 1].rearrange(
                                "p s h -> p s h ()"),
                            in_=wh[:, :, h:h + 1, 0:q],
                            axis=mybir.AxisListType.X, op=OP.add)
                        nc.vector.tensor_reduce(
                            out=sn[:, 0:nb * NS, h:h + 1].rearrange(
                                "p s h -> p s h ()"),
                            in_=wh[:, :, h:h + 1, q:32],
                            axis=mybir.AxisListType.X, op=OP.add)
                    nc.vector.tensor_tensor(
                        out=sc[:, 0:nb * NS], in0=sc[:, 0:nb * NS],
                        in1=sn[:, 0:nb * NS], op=OP.subtract)
                    af = wpool.tile([128, GB * NS, 2], F32, tag="af")
                    s["af"] = af
                    nc.scalar.activation(af[:, 0:nb * NS], sc[:, 0:nb * NS],
                                         AF.Exp)
                    nc.scalar.activation(work[:, 0:nb * NS, HC:HC + 2],
                                         af[:, 0:nb * NS], AF.Copy)

                def stage_y(bi):
                    nb = nb_of(bi)
                    s = st[bi]
                    gA, gB, sxb, w4d = s["gA"], s["gB"], s["sxb"], s["w4d"]
                    af = s["af"]
                    for ti in range(nb):
                        for h in range(2):
                            ab = af[:, ti * NS:(ti + 1) * NS,
                                    h:h + 1].broadcast_to([128, NS, 128])
                            cl, ch = h * 128, h * 128 + 128
                            nc.vector.tensor_tensor(
                                out=w4d[:, ti, 0:TA, cl:ch],
                                in0=gA[:, ti * TA:(ti + 1) * TA, cl:ch],
                                in1=ab[:, 0:TA], op=OP.mult)
                            nc.vector.tensor_tensor(
                                out=w4d[:, ti, TA:TS, cl:ch],
                                in0=gB[:, ti * TB:(ti + 1) * TB, cl:ch],
                                in1=ab[:, TA:TS], op=OP.mult)
                            nc.vector.tensor_tensor(
                                out=w4d[:, ti, TS, cl:ch],
                                in0=sxb[:, ti, 0, cl:ch],
                                in1=ab[:, TS], op=OP.mult)

                def stage_scatter(bi):
                    t0, nb = bi * GB, nb_of(bi)
                    s = st[bi]
                    mk, work = s["mk"], s["work"]
                    pres = []
                    for ti in range(nb):
                        u_ps = psu.tile([128, 258], F32, tag="u")
                        for si in range(NS):
                            nc.tensor.matmul(
                                u_ps[:], mk[:, ti, si * 128:(si + 1) * 128],
                                work[:, ti * NS + si, :],
                                start=(si == 0), stop=(si == NS - 1))
                        pres.append(fin_pre_cb(u_ps))
                    s["pres"] = pres

                def stage_finpost(bi):
                    t0, nb = bi * GB, nb_of(bi)
                    pres = st[bi]["pres"]
                    for ti in range(nb):
                        fin_post_cb(t0 + ti, pres[ti])
                        if ag_cb is not None:
                            ag_cb(t0 + ti)
                    del st[bi]

                # ---- pipelined schedule: gathers dispatched 3 batches
                # ahead, masks/sxb loaded 1-2 ahead, fin writes on ACT-DMA
                for bj in range(min(3, n_batches)):
                    stage_gather(bj)
                stage_load_mkT(0)
                if n_batches > 1:
                    stage_load_mkT(1)
                stage_load_near(0)
                stage_u(0)
                for bi in range(n_batches):
                    if bi + 1 < n_batches:
                        stage_load_near(bi + 1)
                    stage_score(bi)
                    if bi > 0:
                        stage_finpost(bi - 1)
                    if bi + 3 < n_batches:
                        stage_gather(bi + 3)
                    if bi + 2 < n_batches:
                        stage_load_mkT(bi + 2)
                    if bi + 1 < n_batches:
                        stage_u(bi + 1)
                    stage_y(bi)
                    stage_scatter(bi)
                stage_finpost(n_batches - 1)

            # ---------- finalize, split so PE never stalls behind the
            # ACT normalization chain: fin_pre (ACT/DVE, frees u_ps) then
            # fin_post (PE transposes + dense tail) after the next scatters.
            def fin_pre(u_ps, iavT_sb, bT_sb):
                dcol = fpool.tile([128, 2], F32, tag="dcol")
                nc.vector.tensor_scalar(
                    out=dcol[:], in0=u_ps[:, HC:HC + 2],
                    scalar1=epsc_sb[:, 0:1], scalar2=None, op0=OP.add)
                rcol = fpool.tile([128, 2], F32, tag="rcol")
                nc.vector.reciprocal(rcol[:], dcol[:])
                t1 = ftpool.tile([128, 2, 128], BF16, tag="t1")
                for h in range(2):
                    nc.vector.tensor_scalar(
                        out=t1[:, h, :], in0=u_ps[:, h * 128:(h + 1) * 128],
                        scalar1=rcol[:, h:h + 1], scalar2=None, op0=OP.mult)
                return t1

            def fin_tc(t1, iavT_sb, bT_sb):
                pt = psT.tile([128, 2, 128], BF16, tag="fps")
                for h in range(2):
                    nc.tensor.transpose(pt[:, h, :], t1[:, h, :], iden_sb[:])
                cts = []
                for h in range(2):
                    ct = fpool.tile([128, 128], BF16, tag=f"ct{h}")
                    nc.scalar.activation(ct[:], pt[:, h, :], AF.Relu,
                                         scale=iavT_sb[:, h:h + 1],
                                         bias=bT_sb[:, h:h + 1])
                    cts.append(ct)
                return cts

            def fin1_pre(u_ps):
                return fin_pre(u_ps, iavT1_sb, bT1_sb)

            def fin1_post(t, t1):
                cts = fin_tc(t1, iavT1_sb, bT1_sb)
                ot2 = fpool.tile([128, 2, HC], BF16, tag="ot2")
                for j, w2_sb in ((0, wl2_sb), (1, wr2_sb)):
                    pst = psu.tile([128, 258], F32, tag="u")
                    nc.tensor.matmul(pst[:, 0:HC], cts[0][:], w2_sb[:, 0, :],
                                     start=True, stop=False)
                    nc.tensor.matmul(pst[:, 0:HC], cts[1][:], w2_sb[:, 1, :],
                                     start=False, stop=True)
                    nc.scalar.activation(ot2[:, j, :], pst[:, 0:HC], AF.Copy)
                nc.scalar.dma_start(
                    out=loc2[t * 128:(t + 1) * 128, :, :], in_=ot2[:])
                nc.scalar.dma_start(
                    out=own_slice(xl_own2a, xl_own2b, t), in_=ot2[:, 0, :])

            def fin2_pre(u_ps):
                return fin_pre(u_ps, iavT2_sb, bT2_sb)

            def fin2_post(t, t1):
                cts = fin_tc(t1, iavT2_sb, bT2_sb)
                zt_ps = psu.tile([128, 258], F32, tag="u")
                nc.tensor.matmul(zt_ps[:, 0:128], w3_sb[:, 0, :], cts[0][:],
                                 start=True, stop=False)
                nc.tensor.matmul(zt_ps[:, 0:128], w3_sb[:, 1, :], cts[1][:],
                                 start=False, stop=True)
                zt_sb = fpool.tile([128, 128], BF16, tag="ztsb")
                nc.scalar.activation(zt_sb[:], zt_ps[:, 0:128], AF.Identity,
                                     bias=b3c_sb[:, 0:1])
                o_ps = psu.tile([128, 258], F32, tag="u")
                nc.tensor.matmul(o_ps[:, 0:OUT_F], zt_sb[:], w4_sb[:],
                                 start=True, stop=True)
                o_pre = fpool.tile([128, OUT_F], F32, tag="opre")
                nc.vector.scalar_tensor_tensor(
                    out=o_pre[:], in0=o_ps[:, 0:OUT_F], scalar=1.0,
                    in1=b4f_sb[:], op0=OP.mult, op1=OP.add)
                o_sb = fpool.tile([128, OUT_F], F32, tag="osb")
                nc.scalar.activation(o_sb[:], o_pre[:], AF.Sigmoid)
                nc.scalar.dma_start(out=out_ext[t * 128:(t + 1) * 128, :],
                                    in_=o_sb[:])

            # ================= phase schedule =================
            import os as _os
            _upto = int(_os.environ.get("KPHASES", "9"))

            table_local_l1()
            table_full_l1()
            if _upto >= 2:
                ag2 = lambda t: ag_fire((xl_own2a, xl_own2b),
                                        (xl_all2a, xl_all2b), t)
                conv_layer(xl_all1a, xl_all1b, idxXLA1, idxXLB1, loc1,
                           LR1, fin1_pre, fin1_post, ag_cb=ag2)
            if _upto >= 4:
                conv_layer(xl_all2a, xl_all2b, idxXLA, idxXLB, loc2,
                           LR2, fin2_pre, fin2_post)
            else:
                zt = fpool.tile([128, OUT_F], F32, tag="osb")
                nc.vector.memset(zt[:], 0.0)
                for t in range(NTILES):
                    nc.sync.dma_start(out=out_ext[t * 128:(t + 1) * 128, :],
                                      in_=zt[:])

    nc.compile()
    return nc


# ---------------------------------------------------------------- entry point
def kernel(**inputs):
    from concourse import bass_utils

    src = np.asarray(inputs["edge_index"][0], np.int64)
    dst = np.asarray(inputs["edge_index"][1], np.int64)
    x = np.asarray(inputs["x"], np.float32)

    pack = _pack_graph(src, dst)
    nos = pack["node_of_slot"]
    valid = nos >= 0
    x_slot = np.zeros((S, IN_F), np.float32)
    x_slot[valid] = x[nos[valid]]
    sog = pack["slot_of_grow"]
    x_grow = np.zeros((S, IN_F), np.float32)
    gv = sog >= 0
    x_grow[gv] = x_slot[sog[gv]]

    def bf(a):
        return np.ascontiguousarray(np.asarray(a, np.float32)).astype(BF)

    # --- per-head column permutation: sign-pure quads so the fold tree
    # (c, c+32, c+64, c+96 -> final col c) only combines same-sign columns;
    # score = reduce(pos finals) - reduce(neg finals). At most one mixed
    # quad per head; its minority-sign members get flipped on-device.
    def prep_layer(att):
        att = np.asarray(att, np.float32).reshape(2, 128)
        perm = np.zeros(HC, np.int64)
        qs, flips = [], []
        for h in range(2):
            a = att[h]
            pos = list(np.where(a > 0)[0])
            neg = list(np.where(a <= 0)[0])
            p = len(pos)
            np_q = p // 4
            mixed = 1 if p % 4 else 0
            colassign = np.empty((4, 32), np.int64)
            pi = ni = 0
            fl = []
            for j in range(32):
                if j < np_q:
                    for m in range(4):
                        colassign[m, j] = pos[pi]; pi += 1
                elif mixed and j == np_q:
                    for m in range(4):
                        if pi < p:
                            colassign[m, j] = pos[pi]; pi += 1
                        else:
                            colassign[m, j] = neg[ni]; ni += 1
                            fl.append(m * 32 + j)
                else:
                    for m in range(4):
                        colassign[m, j] = neg[ni]; ni += 1
            for m in range(4):
                for j in range(32):
                    perm[h * 128 + m * 32 + j] = h * 128 + colassign[m, j]
            q = np_q + mixed
            assert 0 < q < 32, f"degenerate sign split p={p}"
            qs.append(q)
            flips.append(fl)
        att_p = att.reshape(HC)[perm]
        att_p = np.where(np.abs(att_p) < 1e-30, 1e-30, att_p)
        att_p = np.abs(att_p)
        return perm, att_p, (qs, flips)

    perm1, att1p, LR1 = prep_layer(inputs["att1"])
    perm2, att2p, LR2 = prep_layer(inputs["att2"])
    _LR_RANGES["l1"] = LR1
    _LR_RANGES["l2"] = LR2

    Wl1p = np.asarray(inputs["Wl1"], np.float32)[:, perm1] * att1p[None, :]
    Wr1p = np.asarray(inputs["Wr1"], np.float32)[:, perm1] * att1p[None, :]
    Wl2p = (np.asarray(inputs["Wl2"], np.float32)[perm1][:, perm2]
            * att2p[None, :])
    Wr2p = (np.asarray(inputs["Wr2"], np.float32)[perm1][:, perm2]
            * att2p[None, :])
    W3p = np.asarray(inputs["W3"], np.float32)[perm2]
    b1p = np.asarray(inputs["b1"], np.float32)[perm1]
    b2p = np.asarray(inputs["b2"], np.float32)[perm2]

    common = {
        "wl1": bf(Wl1p), "wr1": bf(Wr1p),
        "wl2": bf(Wl2p), "wr2": bf(Wr2p),
        "w3": bf(W3p), "w4": bf(inputs["W4"]),
        "iavT1": np.ascontiguousarray(
            (1.0 / att1p).reshape(2, 128).T.astype(np.float32)),
        "iavT2": np.ascontiguousarray(
            (1.0 / att2p).reshape(2, 128).T.astype(np.float32)),
        "bT1": np.ascontiguousarray(b1p.reshape(2, 128).T.astype(np.float32)),
        "bT2": np.ascontiguousarray(b2p.reshape(2, 128).T.astype(np.float32)),
        "b3c": np.asarray(inputs["b3"], np.float32).reshape(128, 1),
        "b4f": np.tile(np.asarray(inputs["b4"], np.float32)[None, :], (128, 1)),
        "idenBF": np.eye(128, dtype=np.float32).astype(BF),
        "epsc": np.full((128, 1), 1e-16, np.float32),

        "xgT": np.ascontiguousarray(x_grow.T).astype(BF),
    }

    in_maps = []
    for k in range(NCORES):
        m = dict(common)
        m["xoT"] = np.ascontiguousarray(
            x_slot[k * SPC:(k + 1) * SPC].T).astype(BF)
        ixla = np.empty((NTILES, 128, TA * 8), np.int16)
        ixlb = np.empty((NTILES, 128, TB * 8), np.int16)
        ixla1 = np.empty((NTILES, 128, TA * 8), np.int16)
        ixlb1 = np.empty((NTILES, 128, TB * 8), np.int16)
        for t in range(NTILES):
            ixla[t] = _wrap_idx(pack["idxXL"][k, t, :TA * 128])
            ixlb[t] = _wrap_idx(pack["idxXL"][k, t, TA * 128:])
            ixla1[t] = _wrap_idx(pack["idxXL1"][k, t, :TA * 128])
            ixlb1[t] = _wrap_idx(pack["idxXL1"][k, t, TA * 128:])
        m["idxXLA"] = ixla
        m["idxXLB"] = ixlb
        m["idxXLA1"] = ixla1
        m["idxXLB1"] = ixlb1
        m["mkd"] = pack["mk"][k].astype(BF)
        m["mkTd"] = pack["mkT"][k].astype(BF)
        in_maps.append(m)

    if "nc" not in _NC_CACHE:
        _NC_CACHE["nc"] = _build_nc()
    nc = _NC_CACHE["nc"]

    res = bass_utils.run_bass_kernel_spmd(nc, in_maps,
                                          core_ids=list(range(NCORES)),
                                          **_RUN_OPTS)
    _LAST_RESULTS["res"] = res
    out_slots = np.concatenate([res.results[k]["out"] for k in range(NCORES)], 0)
    return out_slots[pack["slot_of_node"]].astype(np.float32)


# revision 40
# speedup vs baseline: 1.1839x; 1.0216x over previous
"""GATv2 (2-layer, 2-head) Trainium2 kernel, 8-core SPMD — v5.

vs v4: layer-1 xl table computed redundantly on every core (no L1
AllGather, fast startup), conv batches software-pipelined (u-phase of
batch b+1 issues before score/y/scatter of batch b so PE never waits
behind the DVE/ACT chain), DMA loads prefetch 2 batches ahead, leaky
back to single scalar_tensor_tensor, y fully on DVE.
"""
import sys

sys.path.insert(0, "/opt/trn_rl_repo")

import numpy as np
import ml_dtypes

BF = ml_dtypes.bfloat16

# ---- static layout constants (match reference problem sizes) ----
N = 50000
NCORES = 8
LANES = 128
NTILES = 49
SPC = NTILES * LANES          # 6272 slots per core
S = NCORES * SPC              # 50176 total slots
GTILES = S // 128             # 392 gather-table tiles
TA = 7                        # table-A gather subtiles per dst-tile
TB = 7
TS = TA + TB                  # random-edge subtiles (self subtile is extra)
NS = TS + 1                   # subtiles per tile incl self
GB = 3                        # dst-tiles per gather batch
IN_F = 128
HC = 256                      # H*C
OUT_F = 40
SLOPE = 0.2
# AllGather chunking (layer 2 only): 5 tile groups (sum = NTILES).
# Groups 0,1 make up gather table A, groups 2,3,4 table B.
AG_CH = (16, 8, 12, 8, 5)
AG_T0 = (0, 16, 24, 36, 44)
AG_TAB = (0, 0, 1, 1, 1)
NTILES_A = 24                 # tiles in table A
HALFR = NTILES_A * LANES * NCORES   # 24576 rows in table A
_b = [0] * len(AG_CH)
_acc = [0, 0]
for _c in range(len(AG_CH)):
    _b[_c] = (0 if AG_TAB[_c] == 0 else HALFR) + _acc[AG_TAB[_c]]
    _acc[AG_TAB[_c]] += AG_CH[_c] * LANES * NCORES
AG_BASE = tuple(_b)

_NC_CACHE = {}
_RUN_OPTS = {}
_LAST_RESULTS = {}
_LR_RANGES = {}


# ---------------------------------------------------------------- host prep
def _pack_graph(src, dst):
    deg = np.bincount(dst, minlength=N)

    is_self = src == dst
    self_eids = np.full(N, -1, np.int64)
    sids = np.where(is_self)[0]
    self_eids[src[sids]] = sids
    rand_mask = np.ones(len(src), bool)
    rand_mask[self_eids[self_eids >= 0]] = False

    nodes_per_core = (N + NCORES - 1) // NCORES
    order = np.argsort(-deg, kind="stable")
    core_edges = np.zeros(NCORES, np.int64)
    core_nodes = np.zeros(NCORES, np.int64)
    core_of_node = np.full(N, -1, np.int32)
    for v in order:
        k = np.argmin(np.where(core_nodes < nodes_per_core, core_edges, 1 << 60))
        core_of_node[v] = k
        core_edges[k] += deg[v]
        core_nodes[k] += 1

    rsrc, rdst = src[rand_mask], dst[rand_mask]

    # --- chunk-group assignment per core (before tile packing): deal nodes
    # round-robin by out-degree so the gather-table halves stay balanced.
    NG = len(AG_CH)
    odeg = np.bincount(rsrc, minlength=N)
    group_of_node = np.full(N, -1, np.int8)
    gcap = [c * LANES for c in AG_CH]
    for k in range(NCORES):
        vs = np.where(core_of_node == k)[0]
        vs = vs[np.argsort(-odeg[vs], kind="stable")]
        cnt = [0] * NG
        gi = 0
        for v in vs:
            while cnt[gi % NG] >= gcap[gi % NG]:
                gi += 1
            group_of_node[v] = gi % NG
            cnt[gi % NG] += 1
            gi += 1
    eh_node = np.asarray(AG_TAB, np.int8)[group_of_node]

    dA = np.bincount(rdst[eh_node[rsrc] == 0], minlength=N)
    dB = np.bincount(rdst[eh_node[rsrc] == 1], minlength=N)
    capA, capB = TA * LANES, TB * LANES

    tile_of_node = np.full(N, -1, np.int32)
    lane_of_node = np.full(N, -1, np.int32)
    for k in range(NCORES):
        for g in range(NG):
            vs = np.where((core_of_node == k) & (group_of_node == g))[0]
            vs = vs[np.argsort(-(dA[vs] + dB[vs]), kind="stable")]
            nv = len(vs)
            ntg = AG_CH[g]
            tile = np.empty(nv, np.int64)
            for i in range(nv):
                r, c = divmod(i, ntg)
                tile[i] = c if r % 2 == 0 else ntg - 1 - c
            loadA = np.bincount(tile, weights=dA[vs],
                                minlength=ntg).astype(np.int64)
            loadB = np.bincount(tile, weights=dB[vs],
                                minlength=ntg).astype(np.int64)
            it = 0
            while (loadA.max() > capA or loadB.max() > capB) and it < 100000:
                it += 1
                t_bad = int(np.argmax(np.maximum(loadA - capA, loadB - capB)))
                overA = loadA[t_bad] - capA >= loadB[t_bad] - capB
                t_good = int(np.argmin(loadA + loadB))
                in_bad = np.where(tile == t_bad)[0]
                in_good = np.where(tile == t_good)[0]
                d_bad = dA[vs[in_bad]] if overA else dB[vs[in_bad]]
                ib = in_bad[np.argmax(d_bad)]
                ig = in_good[np.argmin(dA[vs[in_good]] + dB[vs[in_good]])]
                for i, frm, to in ((ib, t_bad, t_good), (ig, t_good, t_bad)):
                    v = vs[i]
                    tile[i] = to
                    loadA[frm] -= dA[v]; loadA[to] += dA[v]
                    loadB[frm] -= dB[v]; loadB[to] += dB[v]
            if loadA.max() > capA or loadB.max() > capB:
                raise RuntimeError("edge packing failed; need bigger TA/TB")
            tile_of_node[vs] = AG_T0[g] + tile
            for t in range(ntg):
                nodes_t = vs[tile == t]
                lane_of_node[nodes_t] = np.arange(len(nodes_t))

    slot_of_node = (core_of_node.astype(np.int64) * SPC
                    + tile_of_node * LANES + lane_of_node)
    node_of_slot = np.full(S, -1, np.int64)
    node_of_slot[slot_of_node] = np.arange(N)

    # chunk-major gather-table row of each node
    g_arr = group_of_node.astype(np.int64)
    base = np.asarray(AG_BASE, np.int64)[g_arr]
    t0 = np.asarray(AG_T0, np.int64)[g_arr]
    chw = np.asarray(AG_CH, np.int64)[g_arr]
    grow_of_node = (base + core_of_node * chw * LANES
                    + (tile_of_node - t0) * LANES + lane_of_node)

    srcrow = grow_of_node[rsrc]
    dstslot = slot_of_node[rdst]
    dst_core = (dstslot // SPC).astype(np.int32)
    dst_tile = ((dstslot % SPC) // LANES).astype(np.int32)
    dst_lane = (dstslot % LANES).astype(np.int32)
    eh = (srcrow >= HALFR).astype(np.int8)

    idxXL = np.zeros((NCORES, NTILES, TS * 128), np.int16)
    idxXL1 = np.zeros((NCORES, NTILES, TS * 128), np.int16)

    key = (dst_core.astype(np.int64) * NTILES + dst_tile) * 2 + eh
    es = np.argsort(key, kind="stable")
    ksrc = srcrow[es]; kdl = dst_lane[es]
    kc = dst_core[es]; kt = dst_tile[es]; kh = eh[es]
    gkey = key[es]
    start = np.zeros(len(es), bool)
    start[0] = True
    start[1:] = gkey[1:] != gkey[:-1]
    gs = np.where(start, np.arange(len(es)), 0)
    gidx = np.arange(len(es)) - np.maximum.accumulate(gs)
    off = np.where(kh == 0, 0, TA * 128) + gidx
    tabrow = np.where(kh == 0, ksrc, ksrc - HALFR).astype(np.int64)
    idxXL[kc, kt, off] = tabrow.astype(np.int16)
    # conv1 gathers read the lane-major replicated L1 table:
    # row' = lane*(tiles in table) + tile
    ntt = np.where(kh == 0, HALFR // 128, (S - HALFR) // 128)
    row1 = (tabrow % 128) * ntt + tabrow // 128
    idxXL1[kc, kt, off] = row1.astype(np.int16)

    # one-hot masks: mk [e-lane -> dst-lane] per subtile (incl self at TS),
    # mkT [dst-lane -> e-lane] per random subtile.
    ksi = (off // 128).astype(np.int64)
    kel = (off % 128).astype(np.int64)
    mk = np.zeros((NCORES, NTILES, 128, NS * 128), np.float32)
    mkT = np.zeros((NCORES, NTILES, 128, TS * 128), np.float32)
    mk[kc, kt, kel, ksi * 128 + kdl] = 1.0
    mkT[kc, kt, kdl, ksi * 128 + kel] = 1.0
    vsel = np.where(self_eids >= 0)[0]
    ln = lane_of_node[vsel].astype(np.int64)
    mk[core_of_node[vsel], tile_of_node[vsel], ln, TS * 128 + ln] = 1.0

    # grow-order slot map (for the replicated L1 table build)
    slot_of_grow = np.full(S, -1, np.int64)
    slot_of_grow[grow_of_node[np.arange(N)]] = slot_of_node

    return dict(slot_of_node=slot_of_node, node_of_slot=node_of_slot,
                idxXL=idxXL, idxXL1=idxXL1, mk=mk, mkT=mkT,
                slot_of_grow=slot_of_grow)


def _wrap_idx(idx):
    """[n] -> [128, n//16] wrapped (j at partition j%16, col j//16) + replicated."""
    n = idx.shape[0]
    a = idx.reshape(n // 16, 16).T.astype(np.int16)
    return np.tile(a, (8, 1))


# ---------------------------------------------------------------- device kernel
def _build_nc():
    import concourse.bass as bass
    import concourse.bacc as bacc
    import concourse.tile as tile
    import concourse.mybir as mybir

    F32 = mybir.dt.float32
    BF16 = mybir.dt.bfloat16
    I16 = mybir.dt.int16
    AF = mybir.ActivationFunctionType
    OP = mybir.AluOpType

    LR1, LR2 = _LR_RANGES["l1"], _LR_RANGES["l2"]
    nc = bacc.Bacc(None, target_bir_lowering=False, num_swdge_queues=4)

    # ---- inputs
    xoT = nc.dram_tensor("xoT", [128, SPC], BF16, kind="ExternalInput")
    xgT = nc.dram_tensor("xgT", [128, S], BF16, kind="ExternalInput")
    wl1 = nc.dram_tensor("wl1", [128, HC], BF16, kind="ExternalInput")
    wr1 = nc.dram_tensor("wr1", [128, HC], BF16, kind="ExternalInput")
    wl2 = nc.dram_tensor("wl2", [HC, HC], BF16, kind="ExternalInput")
    wr2 = nc.dram_tensor("wr2", [HC, HC], BF16, kind="ExternalInput")
    w3 = nc.dram_tensor("w3", [HC, 128], BF16, kind="ExternalInput")
    w4 = nc.dram_tensor("w4", [128, OUT_F], BF16, kind="ExternalInput")
    iavT1 = nc.dram_tensor("iavT1", [128, 2], F32, kind="ExternalInput")
    iavT2 = nc.dram_tensor("iavT2", [128, 2], F32, kind="ExternalInput")
    bT1 = nc.dram_tensor("bT1", [128, 2], F32, kind="ExternalInput")
    bT2 = nc.dram_tensor("bT2", [128, 2], F32, kind="ExternalInput")
    b3c = nc.dram_tensor("b3c", [128, 1], F32, kind="ExternalInput")
    b4f = nc.dram_tensor("b4f", [128, OUT_F], F32, kind="ExternalInput")
    idenBF = nc.dram_tensor("idenBF", [128, 128], BF16, kind="ExternalInput")
    epsc = nc.dram_tensor("epsc", [128, 1], F32, kind="ExternalInput")

    idxXLA = nc.dram_tensor("idxXLA", [NTILES, 128, TA * 8], I16,
                            kind="ExternalInput")
    idxXLB = nc.dram_tensor("idxXLB", [NTILES, 128, TB * 8], I16,
                            kind="ExternalInput")
    idxXLA1 = nc.dram_tensor("idxXLA1", [NTILES, 128, TA * 8], I16,
                             kind="ExternalInput")
    idxXLB1 = nc.dram_tensor("idxXLB1", [NTILES, 128, TB * 8], I16,
                             kind="ExternalInput")
    mkd = nc.dram_tensor("mkd", [NTILES, 128, NS * 128], BF16,
                         kind="ExternalInput")
    mkTd = nc.dram_tensor("mkTd", [NTILES, 128, TS * 128], BF16,
                          kind="ExternalInput")
    out_ext = nc.dram_tensor("out", [SPC, OUT_F], F32, kind="ExternalOutput")

    # ---- DRAM intermediates (a/b = gather table split at tile 24)
    RA = NTILES_A * 128           # own rows in table a (3072)
    RB = (NTILES - NTILES_A) * 128  # own rows in table b (3200)
    loc1 = nc.dram_tensor("loc1", [SPC, 2, HC], BF16)
    loc2 = nc.dram_tensor("loc2", [SPC, 2, HC], BF16)
    xl_all1a = nc.dram_tensor("xl_all1a", [HALFR, HC], BF16)
    xl_all1b = nc.dram_tensor("xl_all1b", [S - HALFR, HC], BF16)
    xl_own2a = nc.dram_tensor("xl_own2a", [RA, HC], BF16)
    xl_own2b = nc.dram_tensor("xl_own2b", [RB, HC], BF16)
    xl_all2a = nc.dram_tensor("xl_all2a", [HALFR, HC], BF16,
                              addr_space="Shared")
    xl_all2b = nc.dram_tensor("xl_all2b", [S - HALFR, HC], BF16,
                              addr_space="Shared")

    # per-chunk AllGather metadata
    ag_meta = []
    for c in range(len(AG_CH)):
        t0, nt, tab = AG_T0[c], AG_CH[c], AG_TAB[c]
        own_r0 = (t0 - (0 if tab == 0 else NTILES_A)) * 128
        all_r0 = AG_BASE[c] - (0 if tab == 0 else HALFR)
        ag_meta.append((t0 + nt - 1, tab, own_r0, all_r0, nt * 128))

    with tile.TileContext(nc) as tc:
        with (
            tc.tile_pool(name="const", bufs=1) as cpool,
            tc.tile_pool(name="tabw", bufs=2) as tabw,
            tc.tile_pool(name="ld2", bufs=2) as ld2,
            tc.tile_pool(name="gath", bufs=4) as gpool,
            tc.tile_pool(name="work", bufs=2) as wpool,
            tc.tile_pool(name="fin", bufs=2) as fpool,
            tc.tile_pool(name="fint", bufs=4) as ftpool,
            tc.tile_pool(name="psu", bufs=2, space="PSUM") as psu,
            tc.tile_pool(name="psx", bufs=2, space="PSUM") as psx,
            tc.tile_pool(name="psT", bufs=2, space="PSUM") as psT,
        ):
            # ---------- persistent constants in SBUF
            def load_const(t, shape, dt):
                tl = cpool.tile(shape, dt, tag=t.name, name=t.name + "_sb")
                nc.sync.dma_start(out=tl[:], in_=t[:])
                return tl

            wl1_sb = load_const(wl1, [128, HC], BF16)
            wr1_sb = load_const(wr1, [128, HC], BF16)
            w4_sb = load_const(w4, [128, OUT_F], BF16)
            iavT1_sb = load_const(iavT1, [128, 2], F32)
            iavT2_sb = load_const(iavT2, [128, 2], F32)
            bT1_sb = load_const(bT1, [128, 2], F32)
            bT2_sb = load_const(bT2, [128, 2], F32)
            b3c_sb = load_const(b3c, [128, 1], F32)
            b4f_sb = load_const(b4f, [128, OUT_F], F32)
            iden_sb = load_const(idenBF, [128, 128], BF16)
            epsc_sb = load_const(epsc, [128, 1], F32)


            def load_const2(t, cols, tag):
                tl = cpool.tile([128, 2, cols], BF16, tag=tag, name=tag + "_sb")
                nc.sync.dma_start(
                    out=tl[:], in_=t.rearrange("(a p) c -> p a c", p=128))
                return tl

            wl2_sb = load_const2(wl2, HC, "wl2x")
            wr2_sb = load_const2(wr2, HC, "wr2x")
            w3_sb = load_const2(w3, 128, "w3x")

            def own_slice(owna, ownb, t):
                if t < NTILES_A:
                    return owna[t * 128:(t + 1) * 128, :]
                tb = t - NTILES_A
                return ownb[tb * 128:(tb + 1) * 128, :]

            def all_slice(alla, allb, gt):
                if gt < HALFR // 128:
                    return alla[gt * 128:(gt + 1) * 128, :]
                gb_ = gt - HALFR // 128
                return allb[gb_ * 128:(gb_ + 1) * 128, :]

            def ag_fire(owns, alls, t):
                """Fire any AllGather chunk whose last tile is t."""
                for (lt, tab, own_r0, all_r0, nr) in ag_meta:
                    if lt != t:
                        continue
                    nc.gpsimd.collective_compute(
                        "AllGather", mybir.AluOpType.bypass,
                        replica_groups=[list(range(NCORES))],
                        ins=[owns[tab][own_r0:own_r0 + nr, :]],
                        outs=[alls[tab][all_r0:all_r0 + nr * NCORES, :]])

            # ---------- L1 tables, replicated: every core computes the FULL
            # xl1 gather table from x (no collective), plus its own loc1.
            def table_local_l1():
                for t in range(NTILES):
                    lt = tabw.tile([128, 128], BF16, tag="tablhs")
                    nc.sync.dma_start(out=lt[:],
                                      in_=xoT[:, t * 128:(t + 1) * 128])
                    ot = tabw.tile([128, 2, HC], BF16, tag="tabout")
                    for j, w_sb in ((0, wl1_sb), (1, wr1_sb)):
                        pst = psu.tile([128, 258], F32, tag="u")
                        nc.tensor.matmul(pst[:, 0:HC], lt[:], w_sb[:],
                                         start=True, stop=True)
                        if j == 0:
                            nc.vector.tensor_copy(ot[:, j, :], pst[:, 0:HC])
                        else:
                            nc.scalar.activation(ot[:, j, :], pst[:, 0:HC],
                                                 AF.Copy)
                    nc.scalar.dma_start(
                        out=loc1[t * 128:(t + 1) * 128, :, :], in_=ot[:])

            def table_full_l1():
                TG = 8
                for g0 in range(0, GTILES, TG):
                    lt = tabw.tile([128, TG * 128], BF16, tag="tabghs")
                    nc.sync.dma_start(out=lt[:],
                                      in_=xgT[:, g0 * 128:(g0 + TG) * 128])
                    og = tabw.tile([128, TG, HC], BF16, tag="tabgo")
                    for c in range(2):
                        pst = psx.tile([128, 4, HC], F32, tag="xrb")
                        for j in range(4):
                            nc.tensor.matmul(
                                pst[:, j, :],
                                lt[:, (c * 4 + j) * 128:(c * 4 + j + 1) * 128],
                                wl1_sb[:], start=True, stop=True)
                        if c == 0:
                            nc.vector.tensor_copy(og[:, 0:4, :], pst[:])
                        else:
                            nc.scalar.activation(og[:, 4:8, :], pst[:],
                                                 AF.Copy)
                    # groups of 8 tiles never straddle the A/B split;
                    # lane-major layout: row = p*ntt + t (4 KB contiguous
                    # per partition per group)
                    if g0 < HALFR // 128:
                        dst_t, t0g, ntt = xl_all1a, g0, HALFR // 128
                    else:
                        dst_t, t0g, ntt = (xl_all1b, g0 - HALFR // 128,
                                           (S - HALFR) // 128)
                    nc.sync.dma_start(
                        out=dst_t.rearrange("(p tt) c -> p tt c", p=128)[
                            :, t0g:t0g + TG, :],
                        in_=og[:])

            # ---------- edge phase (one conv layer), software-pipelined
            def conv_layer(xl_ta, xl_tb, ixta, ixtb, loc_tab, qf,
                           fin_pre_cb, fin_post_cb, ag_cb=None):
                n_batches = NTILES // GB + (1 if NTILES % GB else 0)
                st = {}

                def nb_of(bi):
                    return min(bi * GB + GB, NTILES) - bi * GB

                def stage_gather(bi):
                    t0, nb = bi * GB, nb_of(bi)
                    s = st.setdefault(bi, {})
                    ixa = ld2.tile([128, GB, TA * 8], I16, tag="ixa")
                    nc.sync.dma_start(
                        out=ixa[:, 0:nb],
                        in_=ixta[t0:t0 + nb].rearrange("t p c -> p t c"))
                    ixb = ld2.tile([128, GB, TB * 8], I16, tag="ixb")
                    nc.sync.dma_start(
                        out=ixb[:, 0:nb],
                        in_=ixtb[t0:t0 + nb].rearrange("t p c -> p t c"))
                    s["gA"] = gpool.tile([128, GB * TA, HC], BF16, tag="gA", name="gA_sb")
                    s["gB"] = gpool.tile([128, GB * TB, HC], BF16, tag="gB", name="gB_sb")
                    nsa = nb * TA
                    ixa_f = ixa[:, 0:nb].rearrange("p t c -> p (t c)")
                    ixb_f = ixb[:, 0:nb].rearrange("p t c -> p (t c)")
                    for g, ixf, tab, q0 in ((s["gA"], ixa_f, xl_ta, 0),
                                            (s["gB"], ixb_f, xl_tb, 2)):
                        h = nsa // 2
                        nc.gpsimd.dma_gather(
                            out_ap=g[:, 0:h, :], in_ap=tab[:, :],
                            idxs_ap=ixf[:, 0:h * 8],
                            num_idxs=h * 128, num_idxs_reg=h * 128,
                            elem_size=HC, single_packet=False, queue_num=q0)
                        nc.gpsimd.dma_gather(
                            out_ap=g[:, h:nsa, :], in_ap=tab[:, :],
                            idxs_ap=ixf[:, h * 8:nsa * 8],
                            num_idxs=(nsa - h) * 128,
                            num_idxs_reg=(nsa - h) * 128,
                            elem_size=HC, single_packet=False,
                            queue_num=q0 + 1)

                def stage_load_mkT(bi):
                    t0, nb = bi * GB, nb_of(bi)
                    s = st.setdefault(bi, {})
                    s["mkT"] = ld2.tile([128, GB, TS * 128], BF16, tag="mkT", name="mkT_sb")
                    nc.sync.dma_start(
                        out=s["mkT"][:, 0:nb],
                        in_=mkTd[t0:t0 + nb].rearrange("t p c -> p t c"))

                def stage_load_near(bi):
                    t0, nb = bi * GB, nb_of(bi)
                    s = st.setdefault(bi, {})
                    s["sxb"] = ld2.tile([128, GB, 2, HC], BF16, tag="sxb", name="sxb_sb")
                    nc.sync.dma_start(
                        out=s["sxb"][:, 0:nb],
                        in_=loc_tab[t0 * 128:(t0 + nb) * 128].rearrange(
                            "(a p) b c -> p a b c", p=128))
                    s["mk"] = ld2.tile([128, GB, NS * 128], BF16, tag="mk", name="mk_sb")
                    nc.sync.dma_start(
                        out=s["mk"][:, 0:nb],
                        in_=mkd[t0:t0 + nb].rearrange("t p c -> p t c"))

                def stage_u(bi):
                    nb = nb_of(bi)
                    s = st[bi]
                    gA, gB, sxb, mkT = s["gA"], s["gB"], s["sxb"], s["mkT"]
                    work = wpool.tile([128, GB * NS, 258], BF16, tag="work")
                    s["work"] = work
                    w4d = work[:, :, 0:HC].rearrange(
                        "p (t s) c -> p t s c", s=NS)
                    s["w4d"] = w4d
                    # u = xl[src] + xr[dst]: xr via one-hot matmul, xl via
                    # identity-matmul accumulate, ACT drains chunks to bf16.
                    for ti in range(nb):
                        for s0 in range(0, TS, 4):
                            s1 = min(s0 + 4, TS)
                            xrb = psx.tile([128, 4, HC], F32, tag="xrb")
                            for si in range(s0, s1):
                                nc.tensor.matmul(
                                    xrb[:, si - s0, :],
                                    mkT[:, ti, si * 128:(si + 1) * 128],
                                    sxb[:, ti, 1, :], start=True, stop=False)
                                g, gofs = (gA, 0) if si < TA else (gB, TA)
                                nc.tensor.matmul(
                                    xrb[:, si - s0, :], iden_sb[:],
                                    g[:, ti * TA + si - gofs, :],
                                    start=False, stop=True)
                            nc.scalar.activation(
                                w4d[:, ti, s0:s1, :], xrb[:, 0:s1 - s0, :],
                                AF.Prelu, alpha=SLOPE)
                    # self subtile: loc xl + xr, then leaky (max form —
                    # tables are |att|-scaled so every column is max-form)
                    nc.vector.tensor_tensor(
                        out=w4d[:, 0:nb, TS, :],
                        in0=sxb[:, 0:nb, 0, :], in1=sxb[:, 0:nb, 1, :],
                        op=OP.add)
                    nc.vector.scalar_tensor_tensor(
                        out=w4d[:, 0:nb, TS, :], in0=w4d[:, 0:nb, TS, :],
                        scalar=SLOPE, in1=w4d[:, 0:nb, TS, :],
                        op0=OP.mult, op1=OP.max)

                def stage_score(bi):
                    nb = nb_of(bi)
                    s = st[bi]
                    qs, flips = qf
                    work = s["work"]
                    wf = work[:, 0:nb * NS]
                    wh = wf[:, :, 0:HC].rearrange("p s (h c) -> p s h c", h=2)
                    # flip the minority-sign members of each head's single
                    # mixed quad, then the fold tree is sign-pure per column
                    for h in range(2):
                        for c in flips[h]:
                            nc.vector.tensor_scalar(
                                out=wh[:, :, h, c:c + 1],
                                in0=wh[:, :, h, c:c + 1],
                                scalar1=-1.0, scalar2=None, op0=OP.mult)
                    nc.vector.tensor_tensor(
                        out=wh[:, :, :, 0:64], in0=wh[:, :, :, 0:64],
                        in1=wh[:, :, :, 64:128], op=OP.add)
                    nc.vector.tensor_tensor(
                        out=wh[:, :, :, 0:32], in0=wh[:, :, :, 0:32],
                        in1=wh[:, :, :, 32:64], op=OP.add)
                    # score = sum(pos finals) - sum(neg finals) per head
                    sc = wpool.tile([128, GB * NS, 2], F32, tag="sc")
                    sn = wpool.tile([128, GB * NS, 2], F32, tag="sn")
                    for h in range(2):
                        q = qs[h]
                        nc.vector.tensor_reduce(
                            out=sc[:, 0:nb * NS, h:h + 1].rearrange(
                                "p s h -> p s h ()"),
                            in_=wh[:, :, h:h + 1, 0:q],
                            axis=mybir.AxisListType.X, op=OP.add)
                        nc.vector.tensor_reduce(
                            out=sn[:, 0:nb * NS, h:h + 1].rearrange(
                                "p s h -> p s h ()"),
                            in_=wh[:, :, h:h + 1, q:32],
                            axis=mybir.AxisListType.X, op=OP.add)
                    nc.vector.tensor_tensor(
                        out=sc[:, 0:nb * NS], in0=sc[:, 0:nb * NS],
                        in1=sn[:, 0:nb * NS], op=OP.subtract)
                    af = wpool.tile([128, GB * NS, 2], F32, tag="af")
                    s["af"] = af
                    nc.scalar.activation(af[:, 0:nb * NS], sc[:, 0:nb * NS],
                                         AF.Exp)
                    nc.scalar.activation(work[:, 0:nb * NS, HC:HC + 2],
                                         af[:, 0:nb * NS], AF.Copy)

                def stage_y(bi):
                    nb = nb_of(bi)
                    s = st[bi]
                    gA, gB, sxb, w4d = s["gA"], s["gB"], s["sxb"], s["w4d"]
                    af = s["af"]
                    for ti in range(nb):
                        for h in range(2):
                            ab = af[:, ti * NS:(ti + 1) * NS,
                                    h:h + 1].broadcast_to([128, NS, 128])
                            cl, ch = h * 128, h * 128 + 128
                            nc.vector.tensor_tensor(
                                out=w4d[:, ti, 0:TA, cl:ch],
                                in0=gA[:, ti * TA:(ti + 1) * TA, cl:ch],
                                in1=ab[:, 0:TA], op=OP.mult)
                            nc.vector.tensor_tensor(
                                out=w4d[:, ti, TA:TS, cl:ch],
                                in0=gB[:, ti * TB:(ti + 1) * TB, cl:ch],
                                in1=ab[:, TA:TS], op=OP.mult)
                            nc.vector.tensor_tensor(
                                out=w4d[:, ti, TS, cl:ch],
                                in0=sxb[:, ti, 0, cl:ch],
                                in1=ab[:, TS], op=OP.mult)

                def stage_scatter(bi):
                    t0, nb = bi * GB, nb_of(bi)
                    s = st[bi]
                    mk, work = s["mk"], s["work"]
                    pres = []
                    for ti in range(nb):
                        u_ps = psu.tile([128, 258], F32, tag="u")
                        for si in range(NS):
                            nc.tensor.matmul(
                                u_ps[:], mk[:, ti, si * 128:(si + 1) * 128],
                                work[:, ti * NS + si, :],
                                start=(si == 0), stop=(si == NS - 1))
                        pres.append(fin_pre_cb(u_ps))
                    s["pres"] = pres

                def stage_finpost(bi):
                    t0, nb = bi * GB, nb_of(bi)
                    pres = st[bi]["pres"]
                    for ti in range(nb):
                        fin_post_cb(t0 + ti, pres[ti])
                        if ag_cb is not None:
                            ag_cb(t0 + ti)
                    del st[bi]

                # ---- pipelined schedule: gathers dispatched 3 batches
                # ahead, masks/sxb loaded 1-2 ahead, fin writes on ACT-DMA
                for bj in range(min(3, n_batches)):
                    stage_gather(bj)
                stage_load_mkT(0)
                if n_batches > 1:
                    stage_load_mkT(1)
                stage_load_near(0)
                stage_u(0)
                for bi in range(n_batches):
                    if bi + 1 < n_batches:
                        stage_load_near(bi + 1)
                    stage_score(bi)
                    if bi > 0:
                        stage_finpost(bi - 1)
                    if bi + 3 < n_batches:
                        stage_gather(bi + 3)
                    if bi + 2 < n_batches:
                        stage_load_mkT(bi + 2)
                    if bi + 1 < n_batches:
                        stage_u(bi + 1)
                    stage_y(bi)
                    stage_scatter(bi)
                stage_finpost(n_batches - 1)

            # ---------- finalize, split so PE never stalls behind the
            # ACT normalization chain: fin_pre (ACT/DVE, frees u_ps) then
            # fin_post (PE transposes + dense tail) after the next scatters.
            def fin_pre(u_ps, iavT_sb, bT_sb):
                dcol = fpool.tile([128, 2], F32, tag="dcol")
                nc.vector.tensor_scalar(
                    out=dcol[:], in0=u_ps[:, HC:HC + 2],
                    scalar1=epsc_sb[:, 0:1], scalar2=None, op0=OP.add)
                rcol = fpool.tile([128, 2], F32, tag="rcol")
                nc.vector.reciprocal(rcol[:], dcol[:])
                t1 = ftpool.tile([128, 2, 128], BF16, tag="t1")
                for h in range(2):
                    nc.vector.tensor_scalar(
                        out=t1[:, h, :], in0=u_ps[:, h * 128:(h + 1) * 128],
                        scalar1=rcol[:, h:h + 1], scalar2=None, op0=OP.mult)
                return t1

            def fin_tc(t1, iavT_sb, bT_sb):
                pt = psT.tile([128, 2, 128], BF16, tag="fps")
                for h in range(2):
                    nc.tensor.transpose(pt[:, h, :], t1[:, h, :], iden_sb[:])
                cts = []
                for h in range(2):
                    ct = fpool.tile([128, 128], BF16, tag=f"ct{h}")
                    nc.scalar.activation(ct[:], pt[:, h, :], AF.Relu,
                                         scale=iavT_sb[:, h:h + 1],
                                         bias=bT_sb[:, h:h + 1])
                    cts.append(ct)
                return cts

            def fin1_pre(u_ps):
                return fin_pre(u_ps, iavT1_sb, bT1_sb)

            def fin1_post(t, t1):
                cts = fin_tc(t1, iavT1_sb, bT1_sb)
                ot2 = fpool.tile([128, 2, HC], BF16, tag="ot2")
                for j, w2_sb in ((0, wl2_sb), (1, wr2_sb)):
                    pst = psu.tile([128, 258], F32, tag="u")
                    nc.tensor.matmul(pst[:, 0:HC], cts[0][:], w2_sb[:, 0, :],
                                     start=True, stop=False)
                    nc.tensor.matmul(pst[:, 0:HC], cts[1][:], w2_sb[:, 1, :],
                                     start=False, stop=True)
                    nc.scalar.activation(ot2[:, j, :], pst[:, 0:HC], AF.Copy)
                nc.scalar.dma_start(
                    out=loc2[t * 128:(t + 1) * 128, :, :], in_=ot2[:])
                nc.scalar.dma_start(
                    out=own_slice(xl_own2a, xl_own2b, t), in_=ot2[:, 0, :])

            def fin2_pre(u_ps):
                return fin_pre(u_ps, iavT2_sb, bT2_sb)

            def fin2_post(t, t1):
                cts = fin_tc(t1, iavT2_sb, bT2_sb)
                zt_ps = psu.tile([128, 258], F32, tag="u")
                nc.tensor.matmul(zt_ps[:, 0:128], w3_sb[:, 0, :], cts[0][:],
                                 start=True, stop=False)
                nc.tensor.matmul(zt_ps[:, 0:128], w3_sb[:, 1, :], cts[1][:],
                                 start=False, stop=True)
                zt_sb = fpool.tile([128, 128], BF16, tag="ztsb")
                nc.scalar.activation(zt_sb[:], zt_ps[:, 0:128], AF.Identity,
                                     bias=b3c_sb[:, 0:1])
                o_ps = psu.tile([128, 258], F32, tag="u")
                nc.tensor.matmul(o_ps[:, 0:OUT_F], zt_sb[:], w4_sb[:],
                                 start=True, stop=True)
                o_pre = fpool.tile([128, OUT_F], F32, tag="opre")
                nc.vector.scalar_tensor_tensor(
                    out=o_pre[:], in0=o_ps[:, 0:OUT_F], scalar=1.0,
                    in1=b4f_sb[:], op0=OP.mult, op1=OP.add)
                # sigmoid via exp(-x) + reciprocal: keeps ACT on the
                # exp_and_others table set (Sigmoid would force a table
                # reload before every batch's Exp)
                nc.scalar.activation(o_pre[:], o_pre[:], AF.Exp, scale=-1.0)
                nc.vector.tensor_scalar(
                    out=o_pre[:], in0=o_pre[:], scalar1=1.0, scalar2=None,
                    op0=OP.add)
                o_sb = fpool.tile([128, OUT_F], F32, tag="osb")
                nc.vector.reciprocal(o_sb[:], o_pre[:])
                nc.scalar.dma_start(out=out_ext[t * 128:(t + 1) * 128, :],
                                    in_=o_sb[:])

            # ================= phase schedule =================
            import os as _os
            _upto = int(_os.environ.get("KPHASES", "9"))

            table_local_l1()
            table_full_l1()
            if _upto >= 2:
                ag2 = lambda t: ag_fire((xl_own2a, xl_own2b),
                                        (xl_all2a, xl_all2b), t)
                conv_layer(xl_all1a, xl_all1b, idxXLA1, idxXLB1, loc1,
                           LR1, fin1_pre, fin1_post, ag_cb=ag2)
            if _upto >= 4:
                conv_layer(xl_all2a, xl_all2b, idxXLA, idxXLB, loc2,
                           LR2, fin2_pre, fin2_post)
            else:
                zt = fpool.tile([128, OUT_F], F32, tag="osb")
                nc.vector.memset(zt[:], 0.0)
                for t in range(NTILES):
                    nc.sync.dma_start(out=out_ext[t * 128:(t + 1) * 128, :],
                                      in_=zt[:])

    nc.compile()
    return nc


# ---------------------------------------------------------------- entry point
def kernel(**inputs):
    from concourse import bass_utils

    src = np.asarray(inputs["edge_index"][0], np.int64)
    dst = np.asarray(inputs["edge_index"][1], np.int64)
    x = np.asarray(inputs["x"], np.float32)

    pack = _pack_graph(src, dst)
    nos = pack["node_of_slot"]
    valid = nos >= 0
    x_slot = np.zeros((S, IN_F), np.float32)
    x_slot[valid] = x[nos[valid]]
    sog = pack["slot_of_grow"]
    x_grow = np.zeros((S, IN_F), np.float32)
    gv = sog >= 0
    x_grow[gv] = x_slot[sog[gv]]

    def bf(a):
        return np.ascontiguousarray(np.asarray(a, np.float32)).astype(BF)

    # --- per-head column permutation: sign-pure quads so the fold tree
    # (c, c+32, c+64, c+96 -> final col c) only combines same-sign columns;
    # score = reduce(pos finals) - reduce(neg finals). At most one mixed
    # quad per head; its minority-sign members get flipped on-device.
    def prep_layer(att):
        att = np.asarray(att, np.float32).reshape(2, 128)
        perm = np.zeros(HC, np.int64)
        qs, flips = [], []
        for h in range(2):
            a = att[h]
            pos = list(np.where(a > 0)[0])
            neg = list(np.where(a <= 0)[0])
            p = len(pos)
            np_q = p // 4
            mixed = 1 if p % 4 else 0
            colassign = np.empty((4, 32), np.int64)
            pi = ni = 0
            fl = []
            for j in range(32):
                if j < np_q:
                    for m in range(4):
                        colassign[m, j] = pos[pi]; pi += 1
                elif mixed and j == np_q:
                    for m in range(4):
                        if pi < p:
                            colassign[m, j] = pos[pi]; pi += 1
                        else:
                            colassign[m, j] = neg[ni]; ni += 1
                            fl.append(m * 32 + j)
                else:
                    for m in range(4):
                        colassign[m, j] = neg[ni]; ni += 1
            for m in range(4):
                for j in range(32):
                    perm[h * 128 + m * 32 + j] = h * 128 + colassign[m, j]
            q = np_q + mixed
            assert 0 < q < 32, f"degenerate sign split p={p}"
            qs.append(q)
            flips.append(fl)
        att_p = att.reshape(HC)[perm]
        att_p = np.where(np.abs(att_p) < 1e-30, 1e-30, att_p)
        att_p = np.abs(att_p)
        return perm, att_p, (qs, flips)

    perm1, att1p, LR1 = prep_layer(inputs["att1"])
    perm2, att2p, LR2 = prep_layer(inputs["att2"])
    _LR_RANGES["l1"] = LR1
    _LR_RANGES["l2"] = LR2

    Wl1p = np.asarray(inputs["Wl1"], np.float32)[:, perm1] * att1p[None, :]
    Wr1p = np.asarray(inputs["Wr1"], np.float32)[:, perm1] * att1p[None, :]
    Wl2p = (np.asarray(inputs["Wl2"], np.float32)[perm1][:, perm2]
            * att2p[None, :])
    Wr2p = (np.asarray(inputs["Wr2"], np.float32)[perm1][:, perm2]
            * att2p[None, :])
    W3p = np.asarray(inputs["W3"], np.float32)[perm2]
    b1p = np.asarray(inputs["b1"], np.float32)[perm1]
    b2p = np.asarray(inputs["b2"], np.float32)[perm2]

    common = {
        "wl1": bf(Wl1p), "wr1": bf(Wr1p),
        "wl2": bf(Wl2p), "wr2": bf(Wr2p),
        "w3": bf(W3p), "w4": bf(inputs["W4"]),
        "iavT1": np.ascontiguousarray(
            (1.0 / att1p).reshape(2, 128).T.astype(np.float32)),
        "iavT2": np.ascontiguousarray(
            (1.0 / att2p).reshape(2, 128).T.astype(np.float32)),
        "bT1": np.ascontiguousarray(b1p.reshape(2, 128).T.astype(np.float32)),
        "bT2": np.ascontiguousarray(b2p.reshape(2, 128).T.astype(np.float32)),
        "b3c": np.asarray(inputs["b3"], np.float32).reshape(128, 1),
        "b4f": np.tile(np.asarray(inputs["b4"], np.float32)[None, :], (128, 1)),
        "idenBF": np.eye(128, dtype=np.float32).astype(BF),
        "epsc": np.full((128, 1), 1e-16, np.float32),

        "xgT": np.ascontiguousarray(x_grow.T).astype(BF),
    }

    in_maps = []
    for k in range(NCORES):
        m = dict(common)
        m["xoT"] = np.ascontiguousarray(
            x_slot[k * SPC:(k + 1) * SPC].T).astype(BF)
        ixla = np.empty((NTILES, 128, TA * 8), np.int16)
        ixlb = np.empty((NTILES, 128, TB * 8), np.int16)
        ixla1 = np.empty((NTILES, 128, TA * 8), np.int16)
        ixlb1 = np.empty((NTILES, 128, TB * 8), np.int16)
        for t in range(NTILES):
            ixla[t] = _wrap_idx(pack["idxXL"][k, t, :TA * 128])
            ixlb[t] = _wrap_idx(pack["idxXL"][k, t, TA * 128:])
            ixla1[t] = _wrap_idx(pack["idxXL1"][k, t, :TA * 128])
            ixlb1[t] = _wrap_idx(pack["idxXL1"][k, t, TA * 128:])
        m["idxXLA"] = ixla
        m["idxXLB"] = ixlb
        m["idxXLA1"] = ixla1
        m["idxXLB1"] = ixlb1
        m["mkd"] = pack["mk"][k].astype(BF)
        m["mkTd"] = pack["mkT"][k].astype(BF)
        in_maps.append(m)

    if "nc" not in _NC_CACHE:
        _NC_CACHE["nc"] = _build_nc()
    nc = _NC_CACHE["nc"]

    res = bass_utils.run_bass_kernel_spmd(nc, in_maps,
                                          core_ids=list(range(NCORES)),
                                          **_RUN_OPTS)
    _LAST_RESULTS["res"] = res
    out_slots = np.concatenate([res.results[k]["out"] for k in range(NCORES)], 0)
    return out_slots[pack["slot_of_node"]].astype(np.float32)
